# revision 37
# baseline (speedup 1.0000x reference)
"""Trainium2 Bass kernel for nn_Block_67637144977876 (sparse_attention).

Self-contained: accepts FULL inputs, shards across 8 NeuronCores
(data-parallel: core = one batch x one 32-row image band = 4096 tokens
= 16 complete 16x16 windows), runs one SPMD NEFF, gathers the output.

v2 design notes:
 - All per-core inputs are packed into TWO dram blobs (one bf16, one
   f32) to minimize per-call PJRT argument marshalling overhead.
 - The whole block (cross-attn -> window-attn norm/noise -> window attn)
   is fused into ONE pass over eight 512-token chunks with transient
   SBUF tiles; the MLP runs as a second short pass so that the Gelu
   table set is loaded once per repetition instead of per chunk.
 - The pipeline is PURE (DRAM in -> DRAM out, double-buffered
   intermediates), so KREP>1 emits KREP identical back-to-back
   repetitions of the same program: used by test.py to measure the
   steady-state per-forward HW time with dispatch overhead amortized.
 - Attention exp() is evaluated on head PAIRS packed into [128,1024]
   bf16 PSUM banks (halves ScalarE call count); the relative-position
   bias enters as a precomputed exp(bias) multiplier applied on the
   GpSimd engine (frees TensorE from bias-preload matmuls).
 - pixel_norm rsqrt = exp(-0.5*ln(v+eps)) keeps the whole attention
   phase on the natural_log_exp activation table set (a post-compile
   pass retargets/dedups the table loads): 2 table loads per rep
   instead of ~18.
"""
import numpy as np
import ml_dtypes

import concourse.bacc as bacc
import concourse.tile as tile
from concourse import mybir
from concourse.bass_utils import run_bass_kernel_spmd
from concourse.dve_ops import AFFINE_THEN_ADD

F32 = mybir.dt.float32
BF16 = mybir.dt.bfloat16
BF = ml_dtypes.bfloat16

DIM = 256
HEADS = 8
HD = 32
WS = 16
BS = 2
HW = 128
N = HW * HW
EN = 256
HID = 4 * DIM
NCORE = 8
TOK = 4096          # tokens per core
TC = 512            # token chunk (= one window pair)
NTC = TOK // TC
SCALE = HD ** -0.5

# ---- packed blob layouts (columns) ----
# blob16 [128, C16] bf16
O_XT16 = 0                      # [128, 8192]  x bf16: m-chunk at 4096m+512c
O_EMBT = 8192                   # [128, 512]   embT: kc chunk at +256kc
O_W = {n: O_EMBT + 512 + 512 * i for i, n in enumerate(
    ["cqw", "ckw", "cvw", "cpw", "qw", "kw", "vw", "apw"])}  # each [128,512]
O_F1W = O_W["apw"] + 512        # [128, 2048]: kc at +1024kc, hc at +128hc
O_F2W = O_F1W + 2048            # [128, 2048]: hc chunk at +256hc, m at +128m
O_EXPB = O_F2W + 2048           # [128, 8192]: pair (g,kc,p) at 1024*(4g+2kc+p)
O_NROW = O_EXPB + 8192          # row 0 only: [1, 4096] noise*strength (bf16)
C16 = O_NROW + 4096
# blob32 [128, C32] f32
O_XT32 = 0                      # [128, 8192]  x f32
O_BIAS = 8192                   # [128, 16]: cpb m at +m, apb +2+m, f2b +4+m,
C32 = O_BIAS + 16               #            f1b +6+hc (hc in 0..7)

EXPSET = 6      # natural_log_exp_and_others (has both Exp and Ln)
ACT_REMAP = {0: EXPSET, 5: EXPSET}   # exp_and_others / natural_log -> shared

_NC_CACHE = {}


def _rel_pos_index():
    c = np.stack(np.meshgrid(np.arange(WS), np.arange(WS), indexing="ij"))
    c = c.reshape(2, -1)
    rel = c[:, :, None] - c[:, None, :]
    rel = rel.transpose(1, 2, 0) + (WS - 1)
    return rel[..., 0] * (2 * WS - 1) + rel[..., 1]


def _perm():
    """t' (window-major) -> n (row-major within the core's 32x128 slab)."""
    t = np.arange(TOK)
    win, intra = t // 256, t % 256
    wr, wc = win // 8, win % 8
    rr, cc = intra // 16, intra % 16
    return (wr * 16 + rr) * 128 + (wc * 16 + cc)


def _patch_act_tables(nc):
    """Retarget Exp/Ln table loads to the shared natural_log_exp set and
    drop loads that are redundant on the (linear) instruction stream."""
    for b in nc.m.functions[0].blocks:
        insts = b.instructions
        cur = None
        i = 0
        while i < len(insts):
            inst = insts[i]
            if inst.opcode == "LoadActFuncSet":
                want = ACT_REMAP.get(inst.act_func_set_id,
                                     inst.act_func_set_id)
                if want == cur and inst.sync_info is None:
                    del insts[i]
                    continue
                inst.act_func_set_id = want
                cur = want
            i += 1


def build_nc(krep=1):
    nc = bacc.Bacc("TRN2", debug=False)
    blob16 = nc.dram_tensor("blob16", (128, C16), BF16, kind="ExternalInput")
    blob32 = nc.dram_tensor("blob32", (128, C32), F32, kind="ExternalInput")
    outT = nc.dram_tensor("outT", (DIM, TOK), F32, kind="ExternalOutput")

    EXP = mybir.ActivationFunctionType.Exp
    LN = mybir.ActivationFunctionType.Ln
    GELU = mybir.ActivationFunctionType.Gelu

    with tile.TileContext(nc) as tc:
        with (
            tc.tile_pool(name="wts", bufs=1) as wts,
            tc.tile_pool(name="x2p", bufs=2) as x2p,
            tc.tile_pool(name="work", bufs=2) as work,
            tc.tile_pool(name="attn", bufs=2) as attn,
            tc.tile_pool(name="ps", bufs=8, space="PSUM") as ps,
        ):
            def pst(name):
                return ps.tile([128, 512], F32, name=name, tag="bank")

            def wt(shape, name, dtype=F32, bufs=None, tag=None):
                kw_ = {"bufs": bufs} if bufs else {}
                return work.tile(list(shape), dtype, name=name,
                                 tag=tag or name, **kw_)

            # ---- resident weight loads (once, outside the rep loop) ----
            def load16(off, cols, name):
                s = wts.tile([128, cols], BF16, name=name)
                nc.sync.dma_start(out=s, in_=blob16.ap()[:, off:off + cols])
                return s

            s_emb = load16(O_EMBT, 512, "s_emb")
            s_w = {n: load16(o, 512, f"s_{n}") for n, o in O_W.items()}
            s_f1w = load16(O_F1W, 2048, "s_f1w")
            s_f2w = load16(O_F2W, 2048, "s_f2w")
            s_expb = load16(O_EXPB, 8192, "s_expb")
            s_b32 = wts.tile([128, 16], F32, name="s_b32")
            nc.sync.dma_start(out=s_b32, in_=blob32.ap()[:, O_BIAS:O_BIAS + 16])
            s_o32 = wts.tile([128, 32], BF16, name="s_o32")
            nc.vector.memset(s_o32, 1.0)
            s_on = wts.tile([128, 128], BF16, name="s_on")
            nc.vector.memset(s_on, 1.0 / DIM)
            s_eps = wts.tile([128, 1], F32, name="s_eps")
            nc.vector.memset(s_eps, 1e-8)

            def cpb(m):
                return s_b32[:, m:m + 1]

            def apb(m):
                return s_b32[:, 2 + m:3 + m]

            def f2b(m):
                return s_b32[:, 4 + m:5 + m]

            def f1b(hc):
                return s_b32[:, 6 + hc:7 + hc]

            def emit_rep():
                # ---- cross-attn K/V prep (embeddings are an input =>
                #      recompute per rep for honest per-forward timing) ----
                k_cm = [attn.tile([128, EN], BF16, name=f"kcm{m}",
                                  tag=f"kcm{m}") for m in range(2)]
                v_km = [attn.tile([128, DIM], BF16, name=f"vkm{m}",
                                  tag=f"vkm{m}") for m in range(2)]
                for m in range(2):
                    p = pst(f"kvp{m}")
                    for kc in range(2):
                        nc.tensor.matmul(
                            p[:, 0:EN],
                            s_w["ckw"][:, 256 * kc + 128 * m:256 * kc + 128 * (m + 1)],
                            s_emb[:, 256 * kc:256 * (kc + 1)],
                            start=(kc == 0), stop=(kc == 1))
                    nc.vector.tensor_copy(k_cm[m], p[:, 0:EN])
                    p2 = pst(f"vvp{m}")
                    for kc in range(2):
                        nc.tensor.matmul(
                            p2[:, 0:DIM],
                            s_emb[:, 256 * kc + 128 * m:256 * kc + 128 * (m + 1)],
                            s_w["cvw"][:, 256 * kc:256 * (kc + 1)],
                            start=(kc == 0), stop=(kc == 1))
                    nc.vector.tensor_copy(v_km[m], p2[:, 0:DIM])

                x2f = [x2p.tile([128, TOK], BF16, name=f"x2f{m}",
                                tag=f"x2f{m}") for m in range(2)]
                s2b = x2p.tile([128, TOK], BF16, name="s2b", tag="s2b",
                               bufs=1)

                # softmax tail: denominators + AV for one head-group g.
                # epair[kc][p] = [128,1024] bf16 exp tiles (head 2p+j at
                # cols 512j); av_cols(hl, kc) yields (colslice, lhsT) pairs.
                def attn_tail(g, ee, av_cols, pname):
                    sb = pst(f"{pname}sb{g}")
                    for hl in range(4):
                        for kc in range(2):
                            nc.tensor.matmul(
                                sb[32 * hl:32 * hl + 32, :],
                                s_o32[:, 0:32],
                                ee[kc][hl],
                                start=(kc == 0), stop=(kc == 1),
                                tile_position=(0, 32 * hl))
                    rb = wt([128, TC], f"rb{g}", F32, tag=f"rb{g}")
                    nc.vector.reciprocal_approx_fast(out=rb, in_=sb)
                    ou = pst(f"{pname}ou{g}")
                    for hl in range(4):
                        for csl, lhsTs in av_cols(hl):
                            for kc in range(2):
                                nc.tensor.matmul(
                                    ou[32 * hl:32 * hl + 32, csl],
                                    lhsTs[kc],
                                    ee[kc][hl][:, csl],
                                    start=(kc == 0), stop=(kc == 1),
                                    tile_position=(0, 32 * hl))
                    on = attn.tile([128, TC], BF16, name=f"on{g}",
                                   tag=f"on{g}")
                    nc.vector.tensor_mul(on, ou, rb)
                    return on

                # =========== phase A: attention, per 512-token chunk =======
                # software-pipelined: cross(t+1) is emitted before
                # window(t) so TensorE has independent matmuls to run
                # during window(t)'s serial pixel-norm/softmax chains.
                def stage_cross(t):
                    # chunk input loads
                    x0 = [wt([128, TC], f"x0_{m}", F32, tag=f"x0_{m}")
                          for m in range(2)]
                    x16 = [work.tile([128, TC], BF16, name=f"x16_{m}",
                                     tag=f"x16_{m}") for m in range(2)]
                    for m in range(2):
                        nc.sync.dma_start(
                            out=x0[m],
                            in_=blob32.ap()[:, O_XT32 + 4096 * m + TC * t:
                                            O_XT32 + 4096 * m + TC * (t + 1)])
                        nc.sync.dma_start(
                            out=x16[m],
                            in_=blob16.ap()[:, O_XT16 + 4096 * m + TC * t:
                                            O_XT16 + 4096 * m + TC * (t + 1)])

                    # ---- cross attention ----
                    q_cm = [work.tile([128, TC], BF16, name=f"qcm{m}",
                                      tag=f"qcm{m}") for m in range(2)]
                    for m in range(2):
                        p = pst(f"qp{m}")
                        for kc in range(2):
                            nc.tensor.matmul(
                                p,
                                s_w["cqw"][:, 256 * kc + 128 * m:
                                           256 * kc + 128 * (m + 1)],
                                x16[kc], start=(kc == 0), stop=(kc == 1))
                        nc.vector.tensor_copy(q_cm[m], p)
                    o_n = [None, None]
                    for g in range(2):
                        ee = [[None] * 4, [None] * 4]
                        for kc in range(2):
                            scs = []
                            for hl in range(4):
                                sc = pst(f"csc{g}{hl}{kc}")
                                nc.tensor.matmul(
                                    sc,
                                    k_cm[g][32 * hl:32 * hl + 32,
                                            128 * kc:128 * (kc + 1)],
                                    q_cm[g][32 * hl:32 * hl + 32, :],
                                    start=True, stop=True,
                                    tile_position=(32 * hl, 0))
                                scs.append(sc)
                            for hl in range(4):
                                e = attn.tile([128, TC], BF16,
                                              name=f"ce{hl}{kc}",
                                              tag=f"ce{hl}{kc}")
                                nc.scalar.activation(out=e, in_=scs[hl],
                                                     func=EXP)
                                ee[kc][hl] = e

                        def av_cols(hl, g=g):
                            hh = 4 * g + hl
                            return [(slice(0, TC),
                                     [v_km[kc][:, 32 * hh:32 * hh + 32]
                                      for kc in range(2)])]
                        o_n[g] = attn_tail(g, ee, av_cols, f"c{t}")
                    x1 = [wt([128, TC], f"x1_{m}", F32, tag=f"x1_{m}",
                              bufs=3) for m in range(2)]
                    for m in range(2):
                        p = pst(f"cp{m}")
                        for kc in range(2):
                            nc.tensor.matmul(
                                p,
                                s_w["cpw"][:, 256 * kc + 128 * m:
                                           256 * kc + 128 * (m + 1)],
                                o_n[kc], start=(kc == 0), stop=(kc == 1))
                        nc.vector._custom_dve(
                            AFFINE_THEN_ADD, out=x1[m], in0=p,
                            in1=x0[m], s0=1.0, s1=cpb(m))
                    return x1

                def stage_norm(t, x1):
                    nb = wt([128, TC], "nb", BF16, bufs=2)
                    nc.sync.dma_start(
                        out=nb,
                        in_=blob16.ap()[0:1, O_NROW + TC * t:O_NROW + TC * (t + 1)]
                        .to_broadcast([128, TC]))
                    # ---- pixel norm + noise ----
                    sq = [work.tile([128, TC], BF16, name=f"sq{m}",
                                    tag=f"sq{m}") for m in range(2)]
                    for m in range(2):
                        nc.vector.tensor_mul(sq[m], x1[m], x1[m])
                    mb = pst("mb")
                    for kc in range(2):
                        nc.tensor.matmul(mb, s_on, sq[kc],
                                         start=(kc == 0), stop=(kc == 1))
                    rs = wt([128, TC], "rs", F32, bufs=2)
                    nc.scalar.activation(out=rs, in_=mb, func=LN, bias=s_eps)
                    nc.scalar.activation(out=rs, in_=rs, func=EXP, scale=-0.5)
                    xn = [work.tile([128, TC], BF16, name=f"xn{m}",
                                    tag=f"xn{m}") for m in range(2)]
                    for m in range(2):
                        xt = wt([128, TC], f"xt{m}", F32, bufs=2, tag=f"xt{m}")
                        nc.vector.tensor_mul(xt, x1[m], rs)
                        nc.vector.tensor_add(xn[m], xt, nb)
                    return xn

                def stage_winattn(t, x1, xn):
                    tsl = slice(TC * t, TC * (t + 1))
                    # ---- window attention ----
                    qk = {}
                    for m in range(2):
                        for wname in ("qw", "kw"):
                            p = pst(f"qk{wname}{m}")
                            for kc in range(2):
                                nc.tensor.matmul(
                                    p,
                                    s_w[wname][:, 256 * kc + 128 * m:
                                               256 * kc + 128 * (m + 1)],
                                    xn[kc], start=(kc == 0), stop=(kc == 1))
                            d = work.tile([128, TC], BF16, name=f"{wname}{m}",
                                          tag=f"{wname}{m}")
                            nc.vector.tensor_copy(d, p)
                            qk[(wname, m)] = d
                    v_kmw = [[None] * 2 for _ in range(2)]
                    for wloc in range(2):
                        for kcw in range(2):
                            base = 256 * wloc + 128 * kcw
                            p = pst(f"vw{wloc}{kcw}")
                            for cc in range(2):
                                nc.tensor.matmul(
                                    p[:, 0:DIM], xn[cc][:, base:base + 128],
                                    s_w["vw"][:, 256 * cc:256 * (cc + 1)],
                                    start=(cc == 0), stop=(cc == 1))
                            v = attn.tile([128, DIM], BF16,
                                          name=f"vkw{wloc}{kcw}",
                                          tag=f"vkw{wloc}{kcw}")
                            nc.vector.tensor_copy(v, p[:, 0:DIM])
                            v_kmw[wloc][kcw] = v
                    o_n = [None, None]
                    for g in range(2):
                        ee = [[None] * 4, [None] * 4]
                        for kc in range(2):
                            scs = []
                            for hl in range(4):
                                sc = pst(f"wsc{g}{hl}{kc}")
                                for wloc in range(2):
                                    kbase = 256 * wloc + 128 * kc
                                    nc.tensor.matmul(
                                        sc[:, 256 * wloc:256 * (wloc + 1)],
                                        qk[("kw", g)][32 * hl:32 * hl + 32,
                                                      kbase:kbase + 128],
                                        qk[("qw", g)][32 * hl:32 * hl + 32,
                                                      256 * wloc:
                                                      256 * (wloc + 1)],
                                        start=True, stop=True,
                                        tile_position=(32 * hl, 0))
                                scs.append(sc)
                            for hl in range(4):
                                e = attn.tile([128, TC], BF16,
                                              name=f"we{hl}{kc}",
                                              tag=f"we{hl}{kc}")
                                nc.scalar.activation(out=e, in_=scs[hl],
                                                     func=EXP)
                                hh = 4 * g + hl
                                off = 1024 * (4 * g + 2 * kc + (hl // 2)) \
                                    + 512 * (hl % 2)
                                nc.vector.tensor_mul(
                                    e, e, s_expb[:, off:off + 512])
                                ee[kc][hl] = e

                        def av_cols(hl, g=g):
                            hh = 4 * g + hl
                            return [
                                (slice(256 * wloc, 256 * (wloc + 1)),
                                 [v_kmw[wloc][kc][:, 32 * hh:32 * hh + 32]
                                  for kc in range(2)])
                                for wloc in range(2)
                            ]
                        o_n[g] = attn_tail(g, ee, av_cols, f"w{t}")
                    for m in range(2):
                        p = pst(f"ap{m}")
                        for kc in range(2):
                            nc.tensor.matmul(
                                p,
                                s_w["apw"][:, 256 * kc + 128 * m:
                                           256 * kc + 128 * (m + 1)],
                                o_n[kc], start=(kc == 0), stop=(kc == 1))
                        nc.vector._custom_dve(
                            AFFINE_THEN_ADD, out=x2f[m][:, tsl], in0=p,
                            in1=x1[m], s0=1.0, s1=apb(m))

                    # ---- second pixel norm (factor stored for phase B) ----
                    msq = [work.tile([128, TC], BF16, name=f"msq{m}",
                                     tag=f"sq{m}") for m in range(2)]
                    for m in range(2):
                        nc.vector.tensor_mul(msq[m], x2f[m][:, tsl],
                                             x2f[m][:, tsl])
                    mb2 = pst("mb2")
                    for kc in range(2):
                        nc.tensor.matmul(mb2, s_on, msq[kc],
                                         start=(kc == 0), stop=(kc == 1))
                    lnv2 = wt([128, TC], "lnv2", F32, bufs=1)
                    nc.scalar.activation(out=lnv2, in_=mb2, func=LN,
                                         bias=s_eps)
                    nc.scalar.activation(out=s2b[:, tsl], in_=lnv2, func=EXP,
                                         scale=-0.5)

                x1s, xns = {}, {}
                x1s[0] = stage_cross(0)
                x1s[1] = stage_cross(1)
                xns[0] = stage_norm(0, x1s[0])
                for t in range(NTC):
                    if t + 2 < NTC:
                        x1s[t + 2] = stage_cross(t + 2)
                    if t + 1 < NTC:
                        xns[t + 1] = stage_norm(t + 1, x1s[t + 1])
                    stage_winattn(t, x1s[t], xns[t])

                # =========== phase B: MLP, per chunk ===========
                # reversed: chunk NTC-1's inputs are the last ready, so
                # the scheduler cannot hoist its gelus into phase A's ACT
                # stream (which would cost extra table-set reloads).
                for t in reversed(range(NTC)):
                    tsl = slice(TC * t, TC * (t + 1))
                    xn2 = [work.tile([128, TC], BF16, name=f"xn2{m}",
                                     tag=f"xn2{m}") for m in range(2)]
                    for m in range(2):
                        nc.vector.tensor_mul(xn2[m], x2f[m][:, tsl],
                                             s2b[:, tsl])
                    hsb = []
                    for hc in range(8):
                        p = pst(f"f1{hc}")
                        for kc in range(2):
                            nc.tensor.matmul(
                                p,
                                s_f1w[:, 1024 * kc + 128 * hc:
                                      1024 * kc + 128 * (hc + 1)],
                                xn2[kc], start=(kc == 0), stop=(kc == 1))
                        hh = work.tile([128, TC], BF16, name=f"h{hc}",
                                       tag=f"h{hc}", bufs=1)
                        nc.scalar.activation(out=hh, in_=p, func=GELU,
                                             bias=f1b(hc))
                        hsb.append(hh)
                    for m in range(2):
                        p = pst(f"f2{m}")
                        for hc in range(8):
                            nc.tensor.matmul(
                                p,
                                s_f2w[:, 256 * hc + 128 * m:
                                      256 * hc + 128 * (m + 1)],
                                hsb[hc], start=(hc == 0), stop=(hc == 7))
                        xo = wt([128, TC], f"xo{m}", F32, tag=f"xo{m}")
                        nc.vector._custom_dve(
                            AFFINE_THEN_ADD, out=xo, in0=p,
                            in1=x2f[m][:, tsl], s0=1.0, s1=f2b(m))
                        nc.gpsimd.dma_start(
                            out=outT.ap()[128 * m:128 * (m + 1), tsl], in_=xo)

            for _rep in range(krep):
                emit_rep()

    nc.compile()
    _patch_act_tables(nc)
    return nc


def _host_prep(x, embeddings, noise, cq_w, ck_w, cv_w, cp_w, cp_b,
               qkv_w, ap_w, ap_b, rpb_table, noise_strength,
               fc1_w, fc1_b, fc2_w, fc2_b):
    perm = _perm()
    idx = _rel_pos_index()
    bias = np.asarray(rpb_table)[idx.reshape(-1)].reshape(
        WS * WS, WS * WS, HEADS)
    biasT = np.exp(bias.transpose(2, 1, 0))   # exp! [h, key(m), tok(n)]
    expb = np.zeros((128, 8192), np.float32)
    for g in range(2):
        for kc in range(2):
            for p in range(2):
                off = 1024 * (4 * g + 2 * kc + p)
                for j in range(2):
                    h = 4 * g + 2 * p + j
                    blk = biasT[h, 128 * kc:128 * (kc + 1), :]  # [128, 256]
                    expb[:, off + 512 * j:off + 512 * j + 512] = (
                        np.concatenate([blk, blk], axis=1))
    f = np.asarray

    def w2(a):  # [256, X] -> [128, 2X] packed kc chunks side by side
        a = f(a)
        return np.concatenate([a[0:128], a[128:256]], axis=1)

    blob16_shared = np.concatenate([
        w2(f(cq_w) * SCALE), w2(ck_w), w2(cv_w), w2(cp_w),
        w2(f(qkv_w)[:, 0:DIM] * SCALE), w2(f(qkv_w)[:, DIM:2 * DIM]),
        w2(f(qkv_w)[:, 2 * DIM:3 * DIM]), w2(ap_w),
        w2(fc1_w),
        np.concatenate([np.concatenate(
            [f(fc2_w)[128 * hc:128 * (hc + 1), 0:128],
             f(fc2_w)[128 * hc:128 * (hc + 1), 128:256]], axis=1)
            for hc in range(8)], axis=1),
        expb,
    ], axis=1).astype(BF)

    bias32 = np.zeros((128, 16), np.float32)
    bias32[:, 0] = f(cp_b)[0:128]
    bias32[:, 1] = f(cp_b)[128:256]
    bias32[:, 2] = f(ap_b)[0:128]
    bias32[:, 3] = f(ap_b)[128:256]
    bias32[:, 4] = f(fc2_b)[0:128]
    bias32[:, 5] = f(fc2_b)[128:256]
    for hc in range(8):
        bias32[:, 6 + hc] = f(fc1_b)[128 * hc:128 * (hc + 1)]

    ins = []
    for c in range(NCORE):
        b, j = c // 4, c % 4
        xw = np.asarray(x)[b, TOK * j:TOK * (j + 1), :][perm]   # [4096, 256]
        xT = np.ascontiguousarray(xw.T)                          # [256, 4096]
        w0 = 64 * b + 16 * j
        nr = (np.asarray(noise)[w0:w0 + 16, :, 0].reshape(4096)
              * float(noise_strength)).astype(np.float32)
        embT = np.ascontiguousarray(np.asarray(embeddings)[b].T)  # [256, 256]
        nrow16 = np.zeros((128, 4096), BF)
        nrow16[0, :] = nr.astype(BF)
        blob16 = np.concatenate([
            np.concatenate([xT[0:128], xT[128:256]], axis=1).astype(BF),
            np.concatenate([embT[0:128], embT[128:256]], axis=1).astype(BF),
            blob16_shared,
            nrow16,
        ], axis=1)
        blob32 = np.zeros((128, C32), np.float32)
        blob32[:, 0:4096] = xT[0:128]
        blob32[:, 4096:8192] = xT[128:256]
        blob32[:, O_BIAS:O_BIAS + 16] = bias32
        ins.append({"blob16": blob16, "blob32": blob32})
    return ins, perm


def kernel(**inputs):
    global _NC_CACHE
    if 1 not in _NC_CACHE:
        _NC_CACHE[1] = build_nc(1)
    nc = _NC_CACHE[1]
    ins, perm = _host_prep(**inputs)
    res = run_bass_kernel_spmd(nc, ins, core_ids=list(range(NCORE)))
    inv = np.empty(TOK, np.int64)
    inv[perm] = np.arange(TOK)
    out = np.zeros((BS, N, DIM), np.float32)
    for c in range(NCORE):
        b, j = c // 4, c % 4
        oc = res.results[c]["outT"]                  # [256, 4096]
        out[b, TOK * j:TOK * (j + 1), :] = oc.T[inv]
    return out


# revision 38
# speedup vs baseline: 1.1291x; 1.1291x over previous
"""Trainium2 Bass kernel for nn_Block_67637144977876 (sparse_attention).

Self-contained: accepts FULL inputs, shards across 8 NeuronCores
(data-parallel: core = one batch x one 32-row image band = 4096 tokens
= 16 complete 16x16 windows), runs one SPMD NEFF, gathers the output.

v2 design notes:
 - All per-core inputs are packed into TWO dram blobs (one bf16, one
   f32) to minimize per-call PJRT argument marshalling overhead.
 - The whole block (cross-attn -> window-attn norm/noise -> window attn)
   is fused into ONE pass over eight 512-token chunks with transient
   SBUF tiles; the MLP runs as a second short pass so that the Gelu
   table set is loaded once per repetition instead of per chunk.
 - The pipeline is PURE (DRAM in -> DRAM out, double-buffered
   intermediates), so KREP>1 emits KREP identical back-to-back
   repetitions of the same program: used by test.py to measure the
   steady-state per-forward HW time with dispatch overhead amortized.
 - Attention exp() is evaluated on head PAIRS packed into [128,1024]
   bf16 PSUM banks (halves ScalarE call count); the relative-position
   bias enters as a precomputed exp(bias) multiplier applied on the
   GpSimd engine (frees TensorE from bias-preload matmuls).
 - pixel_norm rsqrt = exp(-0.5*ln(v+eps)) keeps the whole attention
   phase on the natural_log_exp activation table set (a post-compile
   pass retargets/dedups the table loads): 2 table loads per rep
   instead of ~18.
"""
import numpy as np
import ml_dtypes

import concourse.bacc as bacc
import concourse.tile as tile
from concourse import mybir
from concourse.bass_utils import run_bass_kernel_spmd
from concourse.dve_ops import AFFINE_THEN_ADD

F32 = mybir.dt.float32
BF16 = mybir.dt.bfloat16
BF = ml_dtypes.bfloat16

DIM = 256
HEADS = 8
HD = 32
WS = 16
BS = 2
HW = 128
N = HW * HW
EN = 256
HID = 4 * DIM
NCORE = 8
TOK = 4096          # tokens per core
TC = 512            # token chunk (= one window pair)
NTC = TOK // TC
SCALE = HD ** -0.5

# ---- packed blob layouts (columns) ----
# blob16 [128, C16] bf16
O_XT16 = 0                      # [128, 8192]  x bf16: m-chunk at 4096m+512c
O_EMBT = 8192                   # [128, 512]   embT: kc chunk at +256kc
O_W = {n: O_EMBT + 512 + 512 * i for i, n in enumerate(
    ["cqw", "ckw", "cvw", "cpw", "qw", "kw", "vw", "apw"])}  # each [128,512]
O_F1W = O_W["apw"] + 512        # [128, 2048]: kc at +1024kc, hc at +128hc
O_F2W = O_F1W + 2048            # [128, 2048]: hc chunk at +256hc, m at +128m
O_EXPB = O_F2W + 2048           # [128, 8192]: pair (g,kc,p) at 1024*(4g+2kc+p)
O_NROW = O_EXPB + 8192          # row 0 only: [1, 4096] noise*strength (bf16)
C16 = O_NROW + 4096
# blob32 [128, C32] f32
O_XT32 = 0                      # [128, 8192]  x f32
O_BIAS = 8192                   # [128, 16]: cpb m at +m, apb +2+m, f2b +4+m,
C32 = O_BIAS + 16               #            f1b +6+hc (hc in 0..7)

EXPSET = 6      # natural_log_exp_and_others (has both Exp and Ln)
ACT_REMAP = {0: EXPSET, 5: EXPSET}   # exp_and_others / natural_log -> shared

_NC_CACHE = {}


def _rel_pos_index():
    c = np.stack(np.meshgrid(np.arange(WS), np.arange(WS), indexing="ij"))
    c = c.reshape(2, -1)
    rel = c[:, :, None] - c[:, None, :]
    rel = rel.transpose(1, 2, 0) + (WS - 1)
    return rel[..., 0] * (2 * WS - 1) + rel[..., 1]


def _perm():
    """t' (window-major) -> n (row-major within the core's 32x128 slab)."""
    t = np.arange(TOK)
    win, intra = t // 256, t % 256
    wr, wc = win // 8, win % 8
    rr, cc = intra // 16, intra % 16
    return (wr * 16 + rr) * 128 + (wc * 16 + cc)


def _patch_act_tables(nc):
    """Retarget Exp/Ln table loads to the shared natural_log_exp set and
    drop loads that are redundant on the (linear) instruction stream."""
    for b in nc.m.functions[0].blocks:
        insts = b.instructions
        cur = None
        i = 0
        while i < len(insts):
            inst = insts[i]
            if inst.opcode == "LoadActFuncSet":
                want = ACT_REMAP.get(inst.act_func_set_id,
                                     inst.act_func_set_id)
                if want == cur and inst.sync_info is None:
                    del insts[i]
                    continue
                inst.act_func_set_id = want
                cur = want
            i += 1


def build_nc(krep=1):
    nc = bacc.Bacc("TRN2", debug=False)
    blob16 = nc.dram_tensor("blob16", (128, C16), BF16, kind="ExternalInput")
    blob32 = nc.dram_tensor("blob32", (128, C32), F32, kind="ExternalInput")
    outT = nc.dram_tensor("outT", (DIM, TOK), F32, kind="ExternalOutput")

    EXP = mybir.ActivationFunctionType.Exp
    LN = mybir.ActivationFunctionType.Ln
    GELU = mybir.ActivationFunctionType.Gelu

    with tile.TileContext(nc) as tc:
        with (
            tc.tile_pool(name="wts", bufs=1) as wts,
            tc.tile_pool(name="x2p", bufs=2) as x2p,
            tc.tile_pool(name="work", bufs=2) as work,
            tc.tile_pool(name="attn", bufs=2) as attn,
            tc.tile_pool(name="ps", bufs=8, space="PSUM") as ps,
        ):
            def pst(name):
                return ps.tile([128, 512], F32, name=name, tag="bank")

            def wt(shape, name, dtype=F32, bufs=None, tag=None):
                kw_ = {"bufs": bufs} if bufs else {}
                return work.tile(list(shape), dtype, name=name,
                                 tag=tag or name, **kw_)

            # ---- resident weight loads (once, outside the rep loop) ----
            def load16(off, cols, name):
                s = wts.tile([128, cols], BF16, name=name)
                nc.sync.dma_start(out=s, in_=blob16.ap()[:, off:off + cols])
                return s

            s_emb = load16(O_EMBT, 512, "s_emb")
            s_w = {n: load16(o, 512, f"s_{n}") for n, o in O_W.items()}
            s_f1w = load16(O_F1W, 2048, "s_f1w")
            s_f2w = load16(O_F2W, 2048, "s_f2w")
            s_expb = load16(O_EXPB, 8192, "s_expb")
            s_b32 = wts.tile([128, 16], F32, name="s_b32")
            nc.sync.dma_start(out=s_b32, in_=blob32.ap()[:, O_BIAS:O_BIAS + 16])
            s_o32 = wts.tile([128, 32], BF16, name="s_o32")
            nc.vector.memset(s_o32, 1.0)
            s_on = wts.tile([128, 128], BF16, name="s_on")
            nc.vector.memset(s_on, 1.0 / DIM)
            s_eps = wts.tile([128, 1], F32, name="s_eps")
            nc.vector.memset(s_eps, 1e-8)

            def cpb(m):
                return s_b32[:, m:m + 1]

            def apb(m):
                return s_b32[:, 2 + m:3 + m]

            def f2b(m):
                return s_b32[:, 4 + m:5 + m]

            def f1b(hc):
                return s_b32[:, 6 + hc:7 + hc]

            def emit_rep():
                # ---- cross-attn K/V prep (embeddings are an input =>
                #      recompute per rep for honest per-forward timing) ----
                k_cm = [attn.tile([128, EN], BF16, name=f"kcm{m}",
                                  tag=f"kcm{m}") for m in range(2)]
                v_km = [attn.tile([128, DIM], BF16, name=f"vkm{m}",
                                  tag=f"vkm{m}") for m in range(2)]
                for m in range(2):
                    p = pst(f"kvp{m}")
                    for kc in range(2):
                        nc.tensor.matmul(
                            p[:, 0:EN],
                            s_w["ckw"][:, 256 * kc + 128 * m:256 * kc + 128 * (m + 1)],
                            s_emb[:, 256 * kc:256 * (kc + 1)],
                            start=(kc == 0), stop=(kc == 1))
                    nc.vector.tensor_copy(k_cm[m], p[:, 0:EN])
                    p2 = pst(f"vvp{m}")
                    for kc in range(2):
                        nc.tensor.matmul(
                            p2[:, 0:DIM],
                            s_emb[:, 256 * kc + 128 * m:256 * kc + 128 * (m + 1)],
                            s_w["cvw"][:, 256 * kc:256 * (kc + 1)],
                            start=(kc == 0), stop=(kc == 1))
                    nc.vector.tensor_copy(v_km[m], p2[:, 0:DIM])

                x2f = [x2p.tile([128, TOK], BF16, name=f"x2f{m}",
                                tag=f"x2f{m}") for m in range(2)]
                s2b = x2p.tile([128, TOK], BF16, name="s2b", tag="s2b",
                               bufs=1)

                # softmax tail: denominators + AV for one head-group g.
                # epair[kc][p] = [128,1024] bf16 exp tiles (head 2p+j at
                # cols 512j); av_cols(hl, kc) yields (colslice, lhsT) pairs.
                def attn_tail(g, ee, av_cols, pname):
                    sb = pst(f"{pname}sb{g}")
                    for hl in range(4):
                        for kc in range(2):
                            nc.tensor.matmul(
                                sb[32 * hl:32 * hl + 32, :],
                                s_o32[:, 0:32],
                                ee[kc][hl],
                                start=(kc == 0), stop=(kc == 1),
                                tile_position=(0, 32 * hl))
                    rb = wt([128, TC], f"rb{g}", F32, tag=f"rb{g}")
                    nc.vector.reciprocal_approx_fast(out=rb, in_=sb)
                    ou = pst(f"{pname}ou{g}")
                    for hl in range(4):
                        for csl, lhsTs in av_cols(hl):
                            for kc in range(2):
                                nc.tensor.matmul(
                                    ou[32 * hl:32 * hl + 32, csl],
                                    lhsTs[kc],
                                    ee[kc][hl][:, csl],
                                    start=(kc == 0), stop=(kc == 1),
                                    tile_position=(0, 32 * hl))
                    on = attn.tile([128, TC], BF16, name=f"on{g}",
                                   tag=f"on{g}")
                    nc.vector.tensor_mul(on, ou, rb)
                    return on

                # =========== phase A: attention, per 512-token chunk =======
                # software-pipelined: cross(t+1) is emitted before
                # window(t) so TensorE has independent matmuls to run
                # during window(t)'s serial pixel-norm/softmax chains.
                def stage_cross(t):
                    # chunk input loads
                    x0 = [wt([128, TC], f"x0_{m}", F32, tag=f"x0_{m}")
                          for m in range(2)]
                    x16 = [work.tile([128, TC], BF16, name=f"x16_{m}",
                                     tag=f"x16_{m}") for m in range(2)]
                    for m in range(2):
                        nc.sync.dma_start(
                            out=x0[m],
                            in_=blob32.ap()[:, O_XT32 + 4096 * m + TC * t:
                                            O_XT32 + 4096 * m + TC * (t + 1)])
                        nc.sync.dma_start(
                            out=x16[m],
                            in_=blob16.ap()[:, O_XT16 + 4096 * m + TC * t:
                                            O_XT16 + 4096 * m + TC * (t + 1)])

                    # ---- cross attention ----
                    q_cm = [work.tile([128, TC], BF16, name=f"qcm{m}",
                                      tag=f"qcm{m}") for m in range(2)]
                    for m in range(2):
                        p = pst(f"qp{m}")
                        for kc in range(2):
                            nc.tensor.matmul(
                                p,
                                s_w["cqw"][:, 256 * kc + 128 * m:
                                           256 * kc + 128 * (m + 1)],
                                x16[kc], start=(kc == 0), stop=(kc == 1))
                        nc.vector.tensor_copy(q_cm[m], p)
                    o_n = [None, None]
                    for g in range(2):
                        ee = [[None] * 4, [None] * 4]
                        for kc in range(2):
                            scs = []
                            for hl in range(4):
                                sc = pst(f"csc{g}{hl}{kc}")
                                nc.tensor.matmul(
                                    sc,
                                    k_cm[g][32 * hl:32 * hl + 32,
                                            128 * kc:128 * (kc + 1)],
                                    q_cm[g][32 * hl:32 * hl + 32, :],
                                    start=True, stop=True,
                                    tile_position=(32 * hl, 0))
                                scs.append(sc)
                            for hl in range(4):
                                e = attn.tile([128, TC], BF16,
                                              name=f"ce{hl}{kc}",
                                              tag=f"ce{hl}{kc}")
                                nc.scalar.activation(out=e, in_=scs[hl],
                                                     func=EXP)
                                ee[kc][hl] = e

                        def av_cols(hl, g=g):
                            hh = 4 * g + hl
                            return [(slice(0, TC),
                                     [v_km[kc][:, 32 * hh:32 * hh + 32]
                                      for kc in range(2)])]
                        o_n[g] = attn_tail(g, ee, av_cols, f"c{t}")
                    x1 = [wt([128, TC], f"x1_{m}", F32, tag=f"x1_{m}",
                              bufs=3) for m in range(2)]
                    for m in range(2):
                        p = pst(f"cp{m}")
                        for kc in range(2):
                            nc.tensor.matmul(
                                p,
                                s_w["cpw"][:, 256 * kc + 128 * m:
                                           256 * kc + 128 * (m + 1)],
                                o_n[kc], start=(kc == 0), stop=(kc == 1))
                        nc.vector._custom_dve(
                            AFFINE_THEN_ADD, out=x1[m], in0=p,
                            in1=x0[m], s0=1.0, s1=cpb(m))
                    return x1

                def stage_norm(t, x1):
                    nb = wt([128, TC], "nb", BF16, bufs=2)
                    nc.sync.dma_start(
                        out=nb,
                        in_=blob16.ap()[0:1, O_NROW + TC * t:O_NROW + TC * (t + 1)]
                        .to_broadcast([128, TC]))
                    # ---- pixel norm + noise ----
                    sq = [work.tile([128, TC], BF16, name=f"sq{m}",
                                    tag=f"sq{m}") for m in range(2)]
                    for m in range(2):
                        nc.vector.tensor_mul(sq[m], x1[m], x1[m])
                    mb = pst("mb")
                    for kc in range(2):
                        nc.tensor.matmul(mb, s_on, sq[kc],
                                         start=(kc == 0), stop=(kc == 1))
                    rs = wt([128, TC], "rs", F32, bufs=2)
                    nc.scalar.activation(out=rs, in_=mb, func=LN, bias=s_eps)
                    nc.scalar.activation(out=rs, in_=rs, func=EXP, scale=-0.5)
                    xn = [work.tile([128, TC], BF16, name=f"xn{m}",
                                    tag=f"xn{m}") for m in range(2)]
                    for m in range(2):
                        xt = wt([128, TC], f"xt{m}", F32, bufs=2, tag=f"xt{m}")
                        nc.vector.tensor_mul(xt, x1[m], rs)
                        nc.vector.tensor_add(xn[m], xt, nb)
                    return xn

                def stage_winattn(t, x1, xn):
                    tsl = slice(TC * t, TC * (t + 1))
                    # ---- window attention ----
                    qk = {}
                    for m in range(2):
                        for wname in ("qw", "kw"):
                            p = pst(f"qk{wname}{m}")
                            for kc in range(2):
                                nc.tensor.matmul(
                                    p,
                                    s_w[wname][:, 256 * kc + 128 * m:
                                               256 * kc + 128 * (m + 1)],
                                    xn[kc], start=(kc == 0), stop=(kc == 1))
                            d = work.tile([128, TC], BF16, name=f"{wname}{m}",
                                          tag=f"{wname}{m}")
                            nc.vector.tensor_copy(d, p)
                            qk[(wname, m)] = d
                    v_kmw = [[None] * 2 for _ in range(2)]
                    for wloc in range(2):
                        for kcw in range(2):
                            base = 256 * wloc + 128 * kcw
                            p = pst(f"vw{wloc}{kcw}")
                            for cc in range(2):
                                nc.tensor.matmul(
                                    p[:, 0:DIM], xn[cc][:, base:base + 128],
                                    s_w["vw"][:, 256 * cc:256 * (cc + 1)],
                                    start=(cc == 0), stop=(cc == 1))
                            v = attn.tile([128, DIM], BF16,
                                          name=f"vkw{wloc}{kcw}",
                                          tag=f"vkw{wloc}{kcw}")
                            nc.vector.tensor_copy(v, p[:, 0:DIM])
                            v_kmw[wloc][kcw] = v
                    o_n = [None, None]
                    for g in range(2):
                        ee = [[None] * 4, [None] * 4]
                        for kc in range(2):
                            scs = []
                            for hl in range(4):
                                sc = pst(f"wsc{g}{hl}{kc}")
                                for wloc in range(2):
                                    kbase = 256 * wloc + 128 * kc
                                    nc.tensor.matmul(
                                        sc[:, 256 * wloc:256 * (wloc + 1)],
                                        qk[("kw", g)][32 * hl:32 * hl + 32,
                                                      kbase:kbase + 128],
                                        qk[("qw", g)][32 * hl:32 * hl + 32,
                                                      256 * wloc:
                                                      256 * (wloc + 1)],
                                        start=True, stop=True,
                                        tile_position=(32 * hl, 0))
                                scs.append(sc)
                            for hl in range(4):
                                e = attn.tile([128, TC], BF16,
                                              name=f"we{hl}{kc}",
                                              tag=f"we{hl}{kc}")
                                nc.scalar.activation(out=e, in_=scs[hl],
                                                     func=EXP)
                                hh = 4 * g + hl
                                off = 1024 * (4 * g + 2 * kc + (hl // 2)) \
                                    + 512 * (hl % 2)
                                nc.vector.tensor_mul(
                                    e, e, s_expb[:, off:off + 512])
                                ee[kc][hl] = e

                        def av_cols(hl, g=g):
                            hh = 4 * g + hl
                            return [
                                (slice(256 * wloc, 256 * (wloc + 1)),
                                 [v_kmw[wloc][kc][:, 32 * hh:32 * hh + 32]
                                  for kc in range(2)])
                                for wloc in range(2)
                            ]
                        o_n[g] = attn_tail(g, ee, av_cols, f"w{t}")
                    for m in range(2):
                        p = pst(f"ap{m}")
                        for kc in range(2):
                            nc.tensor.matmul(
                                p,
                                s_w["apw"][:, 256 * kc + 128 * m:
                                           256 * kc + 128 * (m + 1)],
                                o_n[kc], start=(kc == 0), stop=(kc == 1))
                        nc.vector._custom_dve(
                            AFFINE_THEN_ADD, out=x2f[m][:, tsl], in0=p,
                            in1=x1[m], s0=1.0, s1=apb(m))

                    # ---- second pixel norm (factor stored for phase B) ----
                    msq = [work.tile([128, TC], BF16, name=f"msq{m}",
                                     tag=f"sq{m}") for m in range(2)]
                    for m in range(2):
                        nc.vector.tensor_mul(msq[m], x2f[m][:, tsl],
                                             x2f[m][:, tsl])
                    mb2 = pst("mb2")
                    for kc in range(2):
                        nc.tensor.matmul(mb2, s_on, msq[kc],
                                         start=(kc == 0), stop=(kc == 1))
                    lnv2 = wt([128, TC], "lnv2", F32, bufs=1)
                    nc.scalar.activation(out=lnv2, in_=mb2, func=LN,
                                         bias=s_eps)
                    nc.scalar.activation(out=s2b[:, tsl], in_=lnv2, func=EXP,
                                         scale=-0.5)

                x1s, xns = {}, {}
                x1s[0] = stage_cross(0)
                x1s[1] = stage_cross(1)
                xns[0] = stage_norm(0, x1s[0])
                for t in range(NTC):
                    if t + 2 < NTC:
                        x1s[t + 2] = stage_cross(t + 2)
                    if t + 1 < NTC:
                        xns[t + 1] = stage_norm(t + 1, x1s[t + 1])
                    stage_winattn(t, x1s[t], xns[t])

                # =========== phase B: MLP, per chunk ===========
                for t in range(NTC):
                    tsl = slice(TC * t, TC * (t + 1))
                    xn2 = [work.tile([128, TC], BF16, name=f"xn2{m}",
                                     tag=f"xn2{m}") for m in range(2)]
                    for m in range(2):
                        nc.vector.tensor_mul(xn2[m], x2f[m][:, tsl],
                                             s2b[:, tsl])
                    hsb = []
                    for hc in range(8):
                        p = pst(f"f1{hc}")
                        for kc in range(2):
                            nc.tensor.matmul(
                                p,
                                s_f1w[:, 1024 * kc + 128 * hc:
                                      1024 * kc + 128 * (hc + 1)],
                                xn2[kc], start=(kc == 0), stop=(kc == 1))
                        hh = work.tile([128, TC], BF16, name=f"h{hc}",
                                       tag=f"h{hc}", bufs=1)
                        nc.scalar.activation(out=hh, in_=p, func=GELU,
                                             bias=f1b(hc))
                        hsb.append(hh)
                    for m in range(2):
                        p = pst(f"f2{m}")
                        for hc in range(8):
                            nc.tensor.matmul(
                                p,
                                s_f2w[:, 256 * hc + 128 * m:
                                      256 * hc + 128 * (m + 1)],
                                hsb[hc], start=(hc == 0), stop=(hc == 7))
                        xo = wt([128, TC], f"xo{m}", F32, tag=f"xo{m}")
                        nc.vector._custom_dve(
                            AFFINE_THEN_ADD, out=xo, in0=p,
                            in1=x2f[m][:, tsl], s0=1.0, s1=f2b(m))
                        nc.gpsimd.dma_start(
                            out=outT.ap()[128 * m:128 * (m + 1), tsl], in_=xo)

            for _rep in range(krep):
                emit_rep()

    nc.compile()
    _patch_act_tables(nc)
    return nc


def _host_prep(x, embeddings, noise, cq_w, ck_w, cv_w, cp_w, cp_b,
               qkv_w, ap_w, ap_b, rpb_table, noise_strength,
               fc1_w, fc1_b, fc2_w, fc2_b):
    perm = _perm()
    idx = _rel_pos_index()
    bias = np.asarray(rpb_table)[idx.reshape(-1)].reshape(
        WS * WS, WS * WS, HEADS)
    biasT = np.exp(bias.transpose(2, 1, 0))   # exp! [h, key(m), tok(n)]
    expb = np.zeros((128, 8192), np.float32)
    for g in range(2):
        for kc in range(2):
            for p in range(2):
                off = 1024 * (4 * g + 2 * kc + p)
                for j in range(2):
                    h = 4 * g + 2 * p + j
                    blk = biasT[h, 128 * kc:128 * (kc + 1), :]  # [128, 256]
                    expb[:, off + 512 * j:off + 512 * j + 512] = (
                        np.concatenate([blk, blk], axis=1))
    f = np.asarray

    def w2(a):  # [256, X] -> [128, 2X] packed kc chunks side by side
        a = f(a)
        return np.concatenate([a[0:128], a[128:256]], axis=1)

    blob16_shared = np.concatenate([
        w2(f(cq_w) * SCALE), w2(ck_w), w2(cv_w), w2(cp_w),
        w2(f(qkv_w)[:, 0:DIM] * SCALE), w2(f(qkv_w)[:, DIM:2 * DIM]),
        w2(f(qkv_w)[:, 2 * DIM:3 * DIM]), w2(ap_w),
        w2(fc1_w),
        np.concatenate([np.concatenate(
            [f(fc2_w)[128 * hc:128 * (hc + 1), 0:128],
             f(fc2_w)[128 * hc:128 * (hc + 1), 128:256]], axis=1)
            for hc in range(8)], axis=1),
        expb,
    ], axis=1).astype(BF)

    bias32 = np.zeros((128, 16), np.float32)
    bias32[:, 0] = f(cp_b)[0:128]
    bias32[:, 1] = f(cp_b)[128:256]
    bias32[:, 2] = f(ap_b)[0:128]
    bias32[:, 3] = f(ap_b)[128:256]
    bias32[:, 4] = f(fc2_b)[0:128]
    bias32[:, 5] = f(fc2_b)[128:256]
    for hc in range(8):
        bias32[:, 6 + hc] = f(fc1_b)[128 * hc:128 * (hc + 1)]

    ins = []
    for c in range(NCORE):
        b, j = c // 4, c % 4
        xw = np.asarray(x)[b, TOK * j:TOK * (j + 1), :][perm]   # [4096, 256]
        xT = np.ascontiguousarray(xw.T)                          # [256, 4096]
        w0 = 64 * b + 16 * j
        nr = (np.asarray(noise)[w0:w0 + 16, :, 0].reshape(4096)
              * float(noise_strength)).astype(np.float32)
        embT = np.ascontiguousarray(np.asarray(embeddings)[b].T)  # [256, 256]
        nrow16 = np.zeros((128, 4096), BF)
        nrow16[0, :] = nr.astype(BF)
        blob16 = np.concatenate([
            np.concatenate([xT[0:128], xT[128:256]], axis=1).astype(BF),
            np.concatenate([embT[0:128], embT[128:256]], axis=1).astype(BF),
            blob16_shared,
            nrow16,
        ], axis=1)
        blob32 = np.zeros((128, C32), np.float32)
        blob32[:, 0:4096] = xT[0:128]
        blob32[:, 4096:8192] = xT[128:256]
        blob32[:, O_BIAS:O_BIAS + 16] = bias32
        ins.append({"blob16": blob16, "blob32": blob32})
    return ins, perm


def kernel(**inputs):
    global _NC_CACHE
    if 1 not in _NC_CACHE:
        _NC_CACHE[1] = build_nc(1)
    nc = _NC_CACHE[1]
    ins, perm = _host_prep(**inputs)
    res = run_bass_kernel_spmd(nc, ins, core_ids=list(range(NCORE)))
    inv = np.empty(TOK, np.int64)
    inv[perm] = np.arange(TOK)
    out = np.zeros((BS, N, DIM), np.float32)
    for c in range(NCORE):
        b, j = c // 4, c % 4
        oc = res.results[c]["outT"]                  # [256, 4096]
        out[b, TOK * j:TOK * (j + 1), :] = oc.T[inv]
    return out


# revision 40
# speedup vs baseline: 1.2008x; 1.0635x over previous
"""Trainium2 Bass kernel for nn_Block_67637144977876 (sparse_attention).

Self-contained: accepts FULL inputs, shards across 8 NeuronCores
(data-parallel: core = one batch x one 32-row image band = 4096 tokens
= 16 complete 16x16 windows), runs one SPMD NEFF, gathers the output.

v2 design notes:
 - All per-core inputs are packed into TWO dram blobs (one bf16, one
   f32) to minimize per-call PJRT argument marshalling overhead.
 - The whole block (cross-attn -> window-attn norm/noise -> window attn)
   is fused into ONE pass over eight 512-token chunks with transient
   SBUF tiles; the MLP runs as a second short pass so that the Gelu
   table set is loaded once per repetition instead of per chunk.
 - The pipeline is PURE (DRAM in -> DRAM out, double-buffered
   intermediates), so KREP>1 emits KREP identical back-to-back
   repetitions of the same program: used by test.py to measure the
   steady-state per-forward HW time with dispatch overhead amortized.
 - Attention exp() is evaluated on head PAIRS packed into [128,1024]
   bf16 PSUM banks (halves ScalarE call count); the relative-position
   bias enters as a precomputed exp(bias) multiplier applied on the
   GpSimd engine (frees TensorE from bias-preload matmuls).
 - pixel_norm rsqrt = exp(-0.5*ln(v+eps)) keeps the whole attention
   phase on the natural_log_exp activation table set (a post-compile
   pass retargets/dedups the table loads): 2 table loads per rep
   instead of ~18.
"""
import numpy as np
import ml_dtypes

import concourse.bacc as bacc
import concourse.tile as tile
from concourse import mybir
from concourse.bass_utils import run_bass_kernel_spmd
from concourse.dve_ops import AFFINE_THEN_ADD

F32 = mybir.dt.float32
BF16 = mybir.dt.bfloat16
BF = ml_dtypes.bfloat16

DIM = 256
HEADS = 8
HD = 32
WS = 16
BS = 2
HW = 128
N = HW * HW
EN = 256
HID = 4 * DIM
NCORE = 8
TOK = 4096          # tokens per core
TC = 512            # token chunk (= one window pair)
NTC = TOK // TC
SCALE = HD ** -0.5

# ---- packed blob layouts (columns) ----
# blob16 [128, C16] bf16
O_XT16 = 0                      # [128, 8192]  x bf16: m-chunk at 4096m+512c
O_EMBT = 8192                   # [128, 512]   embT: kc chunk at +256kc
O_W = {n: O_EMBT + 512 + 512 * i for i, n in enumerate(
    ["cqw", "ckw", "cvw", "cpw", "qw", "kw", "vw", "apw"])}  # each [128,512]
O_F1W = O_W["apw"] + 512        # [128, 2048]: kc at +1024kc, hc at +128hc
O_F2W = O_F1W + 2048            # [128, 2048]: hc chunk at +256hc, m at +128m
O_EXPB = O_F2W + 2048           # [128, 8192]: pair (g,kc,p) at 1024*(4g+2kc+p)
O_NROW = O_EXPB + 8192          # row 0 only: [1, 4096] noise*strength (bf16)
C16 = O_NROW + 4096
# blob32 [128, C32] f32
O_XT32 = 0                      # [128, 8192]  x f32
O_BIAS = 8192                   # [128, 16]: cpb m at +m, apb +2+m, f2b +4+m,
C32 = O_BIAS + 16               #            f1b +6+hc (hc in 0..7)

EXPSET = 6      # natural_log_exp_and_others (has both Exp and Ln)
ACT_REMAP = {0: EXPSET, 5: EXPSET}   # exp_and_others / natural_log -> shared

_NC_CACHE = {}


def _rel_pos_index():
    c = np.stack(np.meshgrid(np.arange(WS), np.arange(WS), indexing="ij"))
    c = c.reshape(2, -1)
    rel = c[:, :, None] - c[:, None, :]
    rel = rel.transpose(1, 2, 0) + (WS - 1)
    return rel[..., 0] * (2 * WS - 1) + rel[..., 1]


def _perm():
    """t' (window-major) -> n (row-major within the core's 32x128 slab)."""
    t = np.arange(TOK)
    win, intra = t // 256, t % 256
    wr, wc = win // 8, win % 8
    rr, cc = intra // 16, intra % 16
    return (wr * 16 + rr) * 128 + (wc * 16 + cc)


def _patch_act_tables(nc):
    """Retarget Exp/Ln table loads to the shared natural_log_exp set and
    drop loads that are redundant on the (linear) instruction stream."""
    for b in nc.m.functions[0].blocks:
        insts = b.instructions
        cur = None
        i = 0
        while i < len(insts):
            inst = insts[i]
            if inst.opcode == "LoadActFuncSet":
                want = ACT_REMAP.get(inst.act_func_set_id,
                                     inst.act_func_set_id)
                if want == cur and inst.sync_info is None:
                    del insts[i]
                    continue
                inst.act_func_set_id = want
                cur = want
            i += 1


def build_nc(krep=1):
    nc = bacc.Bacc("TRN2", debug=False)
    blob16 = nc.dram_tensor("blob16", (128, C16), BF16, kind="ExternalInput")
    blob32 = nc.dram_tensor("blob32", (128, C32), F32, kind="ExternalInput")
    outT = nc.dram_tensor("outT", (DIM, TOK), F32, kind="ExternalOutput")

    EXP = mybir.ActivationFunctionType.Exp
    LN = mybir.ActivationFunctionType.Ln
    GELU = mybir.ActivationFunctionType.Gelu

    with tile.TileContext(nc) as tc:
        with (
            tc.tile_pool(name="wts", bufs=1) as wts,
            tc.tile_pool(name="x2p", bufs=2) as x2p,
            tc.tile_pool(name="work", bufs=2) as work,
            tc.tile_pool(name="attn", bufs=2) as attn,
            tc.tile_pool(name="ps", bufs=8, space="PSUM") as ps,
        ):
            def pst(name):
                return ps.tile([128, 512], F32, name=name, tag="bank")

            def wt(shape, name, dtype=F32, bufs=None, tag=None):
                kw_ = {"bufs": bufs} if bufs else {}
                return work.tile(list(shape), dtype, name=name,
                                 tag=tag or name, **kw_)

            # ---- resident weight loads (once, outside the rep loop) ----
            def load16(off, cols, name):
                s = wts.tile([128, cols], BF16, name=name)
                nc.sync.dma_start(out=s, in_=blob16.ap()[:, off:off + cols])
                return s

            s_emb = load16(O_EMBT, 512, "s_emb")
            s_w = {n: load16(o, 512, f"s_{n}") for n, o in O_W.items()}
            s_f1w = load16(O_F1W, 2048, "s_f1w")
            s_f2w = load16(O_F2W, 2048, "s_f2w")
            s_expb = load16(O_EXPB, 8192, "s_expb")
            s_b32 = wts.tile([128, 16], F32, name="s_b32")
            nc.sync.dma_start(out=s_b32, in_=blob32.ap()[:, O_BIAS:O_BIAS + 16])
            s_o32 = wts.tile([128, 32], BF16, name="s_o32")
            nc.vector.memset(s_o32, 1.0)
            s_on = wts.tile([128, 128], BF16, name="s_on")
            nc.vector.memset(s_on, 1.0 / DIM)
            s_eps = wts.tile([128, 1], F32, name="s_eps")
            nc.vector.memset(s_eps, 1e-8)

            def cpb(m):
                return s_b32[:, m:m + 1]

            def apb(m):
                return s_b32[:, 2 + m:3 + m]

            def f2b(m):
                return s_b32[:, 4 + m:5 + m]

            def f1b(hc):
                return s_b32[:, 6 + hc:7 + hc]

            def emit_rep():
                # ---- cross-attn K/V prep (embeddings are an input =>
                #      recompute per rep for honest per-forward timing) ----
                k_cm = [attn.tile([128, EN], BF16, name=f"kcm{m}",
                                  tag=f"kcm{m}") for m in range(2)]
                v_km = [attn.tile([128, DIM], BF16, name=f"vkm{m}",
                                  tag=f"vkm{m}") for m in range(2)]
                for m in range(2):
                    p = pst(f"kvp{m}")
                    for kc in range(2):
                        nc.tensor.matmul(
                            p[:, 0:EN],
                            s_w["ckw"][:, 256 * kc + 128 * m:256 * kc + 128 * (m + 1)],
                            s_emb[:, 256 * kc:256 * (kc + 1)],
                            start=(kc == 0), stop=(kc == 1))
                    nc.vector.tensor_copy(k_cm[m], p[:, 0:EN])
                    p2 = pst(f"vvp{m}")
                    for kc in range(2):
                        nc.tensor.matmul(
                            p2[:, 0:DIM],
                            s_emb[:, 256 * kc + 128 * m:256 * kc + 128 * (m + 1)],
                            s_w["cvw"][:, 256 * kc:256 * (kc + 1)],
                            start=(kc == 0), stop=(kc == 1))
                    nc.vector.tensor_copy(v_km[m], p2[:, 0:DIM])

                x2f = [x2p.tile([128, TOK], BF16, name=f"x2f{m}",
                                tag=f"x2f{m}") for m in range(2)]
                s2b = x2p.tile([128, TOK], BF16, name="s2b", tag="s2b",
                               bufs=1)

                # softmax tail: denominators + AV for one head-group g.
                # epair[kc][p] = [128,1024] bf16 exp tiles (head 2p+j at
                # cols 512j); av_cols(hl, kc) yields (colslice, lhsT) pairs.
                def attn_tail(g, ee, av_cols, pname):
                    sb = pst(f"{pname}sb{g}")
                    for hl in range(4):
                        for kc in range(2):
                            nc.tensor.matmul(
                                sb[32 * hl:32 * hl + 32, :],
                                s_o32[:, 0:32],
                                ee[kc][hl],
                                start=(kc == 0), stop=(kc == 1),
                                tile_position=(0, 32 * hl))
                    rb = wt([128, TC], f"rb{g}", F32, tag=f"rb{g}")
                    nc.vector.reciprocal_approx_fast(out=rb, in_=sb)
                    ou = pst(f"{pname}ou{g}")
                    for hl in range(4):
                        for csl, lhsTs in av_cols(hl):
                            for kc in range(2):
                                nc.tensor.matmul(
                                    ou[32 * hl:32 * hl + 32, csl],
                                    lhsTs[kc],
                                    ee[kc][hl][:, csl],
                                    start=(kc == 0), stop=(kc == 1),
                                    tile_position=(0, 32 * hl))
                    on = attn.tile([128, TC], BF16, name=f"on{g}",
                                   tag=f"on{g}")
                    nc.vector.tensor_mul(on, ou, rb)
                    return on

                # =========== phase A: attention, per 512-token chunk =======
                # software-pipelined: cross(t+1) is emitted before
                # window(t) so TensorE has independent matmuls to run
                # during window(t)'s serial pixel-norm/softmax chains.
                def stage_cross(t):
                    # chunk input loads
                    x0 = [wt([128, TC], f"x0_{m}", F32, tag=f"x0_{m}")
                          for m in range(2)]
                    x16 = [work.tile([128, TC], BF16, name=f"x16_{m}",
                                     tag=f"x16_{m}") for m in range(2)]
                    for m in range(2):
                        nc.sync.dma_start(
                            out=x0[m],
                            in_=blob32.ap()[:, O_XT32 + 4096 * m + TC * t:
                                            O_XT32 + 4096 * m + TC * (t + 1)])
                        nc.sync.dma_start(
                            out=x16[m],
                            in_=blob16.ap()[:, O_XT16 + 4096 * m + TC * t:
                                            O_XT16 + 4096 * m + TC * (t + 1)])

                    # ---- cross attention ----
                    q_cm = [work.tile([128, TC], BF16, name=f"qcm{m}",
                                      tag=f"qcm{m}") for m in range(2)]
                    for m in range(2):
                        p = pst(f"qp{m}")
                        for kc in range(2):
                            nc.tensor.matmul(
                                p,
                                s_w["cqw"][:, 256 * kc + 128 * m:
                                           256 * kc + 128 * (m + 1)],
                                x16[kc], start=(kc == 0), stop=(kc == 1))
                        nc.vector.tensor_copy(q_cm[m], p)
                    o_n = [None, None]
                    for g in range(2):
                        ee = [[None] * 4, [None] * 4]
                        for kc in range(2):
                            scs = []
                            for hl in range(4):
                                sc = pst(f"csc{g}{hl}{kc}")
                                nc.tensor.matmul(
                                    sc,
                                    k_cm[g][32 * hl:32 * hl + 32,
                                            128 * kc:128 * (kc + 1)],
                                    q_cm[g][32 * hl:32 * hl + 32, :],
                                    start=True, stop=True,
                                    tile_position=(32 * hl, 0))
                                scs.append(sc)
                            for hl in range(4):
                                e = attn.tile([128, TC], BF16,
                                              name=f"ce{hl}{kc}",
                                              tag=f"ce{hl}{kc}")
                                nc.scalar.activation(out=e, in_=scs[hl],
                                                     func=EXP)
                                ee[kc][hl] = e

                        def av_cols(hl, g=g):
                            hh = 4 * g + hl
                            return [(slice(0, TC),
                                     [v_km[kc][:, 32 * hh:32 * hh + 32]
                                      for kc in range(2)])]
                        o_n[g] = attn_tail(g, ee, av_cols, f"c{t}")
                    x1 = [wt([128, TC], f"x1_{m}", F32, tag=f"x1_{m}",
                              bufs=3) for m in range(2)]
                    for m in range(2):
                        p = pst(f"cp{m}")
                        for kc in range(2):
                            nc.tensor.matmul(
                                p,
                                s_w["cpw"][:, 256 * kc + 128 * m:
                                           256 * kc + 128 * (m + 1)],
                                o_n[kc], start=(kc == 0), stop=(kc == 1))
                        nc.vector._custom_dve(
                            AFFINE_THEN_ADD, out=x1[m], in0=p,
                            in1=x0[m], s0=1.0, s1=cpb(m))
                    return x1

                def stage_norm(t, x1):
                    nb = wt([128, TC], "nb", BF16, bufs=2)
                    nc.sync.dma_start(
                        out=nb,
                        in_=blob16.ap()[0:1, O_NROW + TC * t:O_NROW + TC * (t + 1)]
                        .to_broadcast([128, TC]))
                    # ---- pixel norm + noise ----
                    sq = [work.tile([128, TC], BF16, name=f"sq{m}",
                                    tag=f"sq{m}") for m in range(2)]
                    for m in range(2):
                        nc.vector.tensor_mul(sq[m], x1[m], x1[m])
                    mb = pst("mb")
                    for kc in range(2):
                        nc.tensor.matmul(mb, s_on, sq[kc],
                                         start=(kc == 0), stop=(kc == 1))
                    rs = wt([128, TC], "rs", F32, bufs=2)
                    nc.scalar.activation(out=rs, in_=mb, func=LN, bias=s_eps)
                    nc.scalar.activation(out=rs, in_=rs, func=EXP, scale=-0.5)
                    xn = [work.tile([128, TC], BF16, name=f"xn{m}",
                                    tag=f"xn{m}") for m in range(2)]
                    for m in range(2):
                        xt = wt([128, TC], f"xt{m}", F32, bufs=2, tag=f"xt{m}")
                        nc.vector.tensor_mul(xt, x1[m], rs)
                        nc.vector.tensor_add(xn[m], xt, nb)
                    return xn

                def stage_winattn(t, x1, xn):
                    tsl = slice(TC * t, TC * (t + 1))
                    # ---- window attention ----
                    qk = {}
                    for m in range(2):
                        for wname in ("qw", "kw"):
                            p = pst(f"qk{wname}{m}")
                            for kc in range(2):
                                nc.tensor.matmul(
                                    p,
                                    s_w[wname][:, 256 * kc + 128 * m:
                                               256 * kc + 128 * (m + 1)],
                                    xn[kc], start=(kc == 0), stop=(kc == 1))
                            d = work.tile([128, TC], BF16, name=f"{wname}{m}",
                                          tag=f"{wname}{m}")
                            nc.vector.tensor_copy(d, p)
                            qk[(wname, m)] = d
                    v_kmw = [[None] * 2 for _ in range(2)]
                    for wloc in range(2):
                        for kcw in range(2):
                            base = 256 * wloc + 128 * kcw
                            p = pst(f"vw{wloc}{kcw}")
                            for cc in range(2):
                                nc.tensor.matmul(
                                    p[:, 0:DIM], xn[cc][:, base:base + 128],
                                    s_w["vw"][:, 256 * cc:256 * (cc + 1)],
                                    start=(cc == 0), stop=(cc == 1))
                            v = attn.tile([128, DIM], BF16,
                                          name=f"vkw{wloc}{kcw}",
                                          tag=f"vkw{wloc}{kcw}")
                            nc.vector.tensor_copy(v, p[:, 0:DIM])
                            v_kmw[wloc][kcw] = v
                    o_n = [None, None]
                    for g in range(2):
                        ee = [[None] * 4, [None] * 4]
                        for kc in range(2):
                            scs = []
                            for hl in range(4):
                                sc = pst(f"wsc{g}{hl}{kc}")
                                for wloc in range(2):
                                    kbase = 256 * wloc + 128 * kc
                                    nc.tensor.matmul(
                                        sc[:, 256 * wloc:256 * (wloc + 1)],
                                        qk[("kw", g)][32 * hl:32 * hl + 32,
                                                      kbase:kbase + 128],
                                        qk[("qw", g)][32 * hl:32 * hl + 32,
                                                      256 * wloc:
                                                      256 * (wloc + 1)],
                                        start=True, stop=True,
                                        tile_position=(32 * hl, 0))
                                scs.append(sc)
                            for hl in range(4):
                                e = attn.tile([128, TC], BF16,
                                              name=f"we{hl}{kc}",
                                              tag=f"we{hl}{kc}")
                                nc.scalar.activation(out=e, in_=scs[hl],
                                                     func=EXP)
                                hh = 4 * g + hl
                                off = 1024 * (4 * g + 2 * kc + (hl // 2)) \
                                    + 512 * (hl % 2)
                                nc.vector.tensor_mul(
                                    e, e, s_expb[:, off:off + 512])
                                ee[kc][hl] = e

                        def av_cols(hl, g=g):
                            hh = 4 * g + hl
                            return [
                                (slice(256 * wloc, 256 * (wloc + 1)),
                                 [v_kmw[wloc][kc][:, 32 * hh:32 * hh + 32]
                                  for kc in range(2)])
                                for wloc in range(2)
                            ]
                        o_n[g] = attn_tail(g, ee, av_cols, f"w{t}")
                    for m in range(2):
                        p = pst(f"ap{m}")
                        for kc in range(2):
                            nc.tensor.matmul(
                                p,
                                s_w["apw"][:, 256 * kc + 128 * m:
                                           256 * kc + 128 * (m + 1)],
                                o_n[kc], start=(kc == 0), stop=(kc == 1))
                        nc.vector._custom_dve(
                            AFFINE_THEN_ADD, out=x2f[m][:, tsl], in0=p,
                            in1=x1[m], s0=1.0, s1=apb(m))

                    # ---- second pixel norm (factor stored for phase B) ----
                    msq = [work.tile([128, TC], BF16, name=f"msq{m}",
                                     tag=f"sq{m}") for m in range(2)]
                    for m in range(2):
                        nc.vector.tensor_mul(msq[m], x2f[m][:, tsl],
                                             x2f[m][:, tsl])
                    mb2 = pst("mb2")
                    for kc in range(2):
                        nc.tensor.matmul(mb2, s_on, msq[kc],
                                         start=(kc == 0), stop=(kc == 1))
                    lnv2 = wt([128, TC], "lnv2", F32, bufs=1)
                    nc.scalar.activation(out=lnv2, in_=mb2, func=LN,
                                         bias=s_eps)
                    nc.scalar.activation(out=s2b[:, tsl], in_=lnv2, func=EXP,
                                         scale=-0.5)

                x1s, xns = {}, {}
                x1s[0] = stage_cross(0)
                x1s[1] = stage_cross(1)
                xns[0] = stage_norm(0, x1s[0])
                for t in range(NTC):
                    if t + 2 < NTC:
                        x1s[t + 2] = stage_cross(t + 2)
                    if t + 1 < NTC:
                        xns[t + 1] = stage_norm(t + 1, x1s[t + 1])
                    stage_winattn(t, x1s[t], xns[t])

                # =========== phase B: MLP, per chunk ===========
                # reversed: chunk NTC-1's inputs are the last ready, so
                # the scheduler cannot hoist its gelus into phase A's ACT
                # stream (which would cost extra table-set reloads).
                for t in reversed(range(NTC)):
                    tsl = slice(TC * t, TC * (t + 1))
                    xn2 = [work.tile([128, TC], BF16, name=f"xn2{m}",
                                     tag=f"xn2{m}") for m in range(2)]
                    for m in range(2):
                        nc.vector.tensor_mul(xn2[m], x2f[m][:, tsl],
                                             s2b[:, tsl])
                    hsb = []
                    for hc in range(8):
                        p = pst(f"f1{hc}")
                        for kc in range(2):
                            nc.tensor.matmul(
                                p,
                                s_f1w[:, 1024 * kc + 128 * hc:
                                      1024 * kc + 128 * (hc + 1)],
                                xn2[kc], start=(kc == 0), stop=(kc == 1))
                        hh = work.tile([128, TC], BF16, name=f"h{hc}",
                                       tag=f"h{hc}", bufs=1)
                        nc.scalar.activation(out=hh, in_=p, func=GELU,
                                             bias=f1b(hc))
                        hsb.append(hh)
                    for m in range(2):
                        p = pst(f"f2{m}")
                        for hc in range(8):
                            nc.tensor.matmul(
                                p,
                                s_f2w[:, 256 * hc + 128 * m:
                                      256 * hc + 128 * (m + 1)],
                                hsb[hc], start=(hc == 0), stop=(hc == 7))
                        xo = wt([128, TC], f"xo{m}", F32, tag=f"xo{m}")
                        nc.vector._custom_dve(
                            AFFINE_THEN_ADD, out=xo, in0=p,
                            in1=x2f[m][:, tsl], s0=1.0, s1=f2b(m))
                        nc.gpsimd.dma_start(
                            out=outT.ap()[128 * m:128 * (m + 1), tsl], in_=xo)

            for _rep in range(krep):
                emit_rep()

    nc.compile()
    _patch_act_tables(nc)
    return nc


def _host_prep(x, embeddings, noise, cq_w, ck_w, cv_w, cp_w, cp_b,
               qkv_w, ap_w, ap_b, rpb_table, noise_strength,
               fc1_w, fc1_b, fc2_w, fc2_b):
    perm = _perm()
    idx = _rel_pos_index()
    bias = np.asarray(rpb_table)[idx.reshape(-1)].reshape(
        WS * WS, WS * WS, HEADS)
    biasT = np.exp(bias.transpose(2, 1, 0))   # exp! [h, key(m), tok(n)]
    expb = np.zeros((128, 8192), np.float32)
    for g in range(2):
        for kc in range(2):
            for p in range(2):
                off = 1024 * (4 * g + 2 * kc + p)
                for j in range(2):
                    h = 4 * g + 2 * p + j
                    blk = biasT[h, 128 * kc:128 * (kc + 1), :]  # [128, 256]
                    expb[:, off + 512 * j:off + 512 * j + 512] = (
                        np.concatenate([blk, blk], axis=1))
    f = np.asarray

    def w2(a):  # [256, X] -> [128, 2X] packed kc chunks side by side
        a = f(a)
        return np.concatenate([a[0:128], a[128:256]], axis=1)

    blob16_shared = np.concatenate([
        w2(f(cq_w) * SCALE), w2(ck_w), w2(cv_w), w2(cp_w),
        w2(f(qkv_w)[:, 0:DIM] * SCALE), w2(f(qkv_w)[:, DIM:2 * DIM]),
        w2(f(qkv_w)[:, 2 * DIM:3 * DIM]), w2(ap_w),
        w2(fc1_w),
        np.concatenate([np.concatenate(
            [f(fc2_w)[128 * hc:128 * (hc + 1), 0:128],
             f(fc2_w)[128 * hc:128 * (hc + 1), 128:256]], axis=1)
            for hc in range(8)], axis=1),
        expb,
    ], axis=1).astype(BF)

    bias32 = np.zeros((128, 16), np.float32)
    bias32[:, 0] = f(cp_b)[0:128]
    bias32[:, 1] = f(cp_b)[128:256]
    bias32[:, 2] = f(ap_b)[0:128]
    bias32[:, 3] = f(ap_b)[128:256]
    bias32[:, 4] = f(fc2_b)[0:128]
    bias32[:, 5] = f(fc2_b)[128:256]
    for hc in range(8):
        bias32[:, 6 + hc] = f(fc1_b)[128 * hc:128 * (hc + 1)]

    ins = []
    for c in range(NCORE):
        b, j = c // 4, c % 4
        xw = np.asarray(x)[b, TOK * j:TOK * (j + 1), :][perm]   # [4096, 256]
        xT = np.ascontiguousarray(xw.T)                          # [256, 4096]
        w0 = 64 * b + 16 * j
        nr = (np.asarray(noise)[w0:w0 + 16, :, 0].reshape(4096)
              * float(noise_strength)).astype(np.float32)
        embT = np.ascontiguousarray(np.asarray(embeddings)[b].T)  # [256, 256]
        nrow16 = np.zeros((128, 4096), BF)
        nrow16[0, :] = nr.astype(BF)
        blob16 = np.concatenate([
            np.concatenate([xT[0:128], xT[128:256]], axis=1).astype(BF),
            np.concatenate([embT[0:128], embT[128:256]], axis=1).astype(BF),
            blob16_shared,
            nrow16,
        ], axis=1)
        blob32 = np.zeros((128, C32), np.float32)
        blob32[:, 0:4096] = xT[0:128]
        blob32[:, 4096:8192] = xT[128:256]
        blob32[:, O_BIAS:O_BIAS + 16] = bias32
        ins.append({"blob16": blob16, "blob32": blob32})
    return ins, perm


def kernel(**inputs):
    global _NC_CACHE
    if 1 not in _NC_CACHE:
        _NC_CACHE[1] = build_nc(1)
    nc = _NC_CACHE[1]
    ins, perm = _host_prep(**inputs)
    res = run_bass_kernel_spmd(nc, ins, core_ids=list(range(NCORE)))
    inv = np.empty(TOK, np.int64)
    inv[perm] = np.arange(TOK)
    out = np.zeros((BS, N, DIM), np.float32)
    for c in range(NCORE):
        b, j = c // 4, c % 4
        oc = res.results[c]["outT"]                  # [256, 4096]
        out[b, TOK * j:TOK * (j + 1), :] = oc.T[inv]
    return out


# revision 43
# speedup vs baseline: 1.2408x; 1.0333x over previous
"""Trainium2 Bass kernel for nn_Block_67637144977876 (sparse_attention).

Self-contained: accepts FULL inputs, shards across 8 NeuronCores
(data-parallel: core = one batch x one 32-row image band = 4096 tokens
= 16 complete 16x16 windows), runs one SPMD NEFF, gathers the output.

v2 design notes:
 - All per-core inputs are packed into TWO dram blobs (one bf16, one
   f32) to minimize per-call PJRT argument marshalling overhead.
 - The whole block (cross-attn -> window-attn norm/noise -> window attn)
   is fused into ONE pass over eight 512-token chunks with transient
   SBUF tiles; the MLP runs as a second short pass so that the Gelu
   table set is loaded once per repetition instead of per chunk.
 - The pipeline is PURE (DRAM in -> DRAM out, double-buffered
   intermediates), so KREP>1 emits KREP identical back-to-back
   repetitions of the same program: used by test.py to measure the
   steady-state per-forward HW time with dispatch overhead amortized.
 - Attention exp() is evaluated on head PAIRS packed into [128,1024]
   bf16 PSUM banks (halves ScalarE call count); the relative-position
   bias enters as a precomputed exp(bias) multiplier applied on the
   GpSimd engine (frees TensorE from bias-preload matmuls).
 - pixel_norm rsqrt = exp(-0.5*ln(v+eps)) keeps the whole attention
   phase on the natural_log_exp activation table set (a post-compile
   pass retargets/dedups the table loads): 2 table loads per rep
   instead of ~18.
"""
import numpy as np
import ml_dtypes

import concourse.bacc as bacc
import concourse.tile as tile
from concourse import mybir
from concourse.bass_utils import run_bass_kernel_spmd
from concourse.dve_ops import AFFINE_THEN_ADD

F32 = mybir.dt.float32
BF16 = mybir.dt.bfloat16
BF = ml_dtypes.bfloat16

DIM = 256
HEADS = 8
HD = 32
WS = 16
BS = 2
HW = 128
N = HW * HW
EN = 256
HID = 4 * DIM
NCORE = 8
TOK = 4096          # tokens per core
TC = 512            # token chunk (= one window pair)
NTC = TOK // TC
SCALE = HD ** -0.5

# ---- packed blob layouts (columns) ----
# blob16 [128, C16] bf16
O_XT16 = 0                      # [128, 8192]  x bf16: m-chunk at 4096m+512c
O_EMBT = 8192                   # [128, 512]   embT: kc chunk at +256kc
O_W = {n: O_EMBT + 512 + 512 * i for i, n in enumerate(
    ["cqw", "ckw", "cvw", "cpw", "qw", "kw", "vw", "apw"])}  # each [128,512]
O_F1W = O_W["apw"] + 512        # [128, 2048]: kc at +1024kc, hc at +128hc
O_F2W = O_F1W + 2048            # [128, 2048]: hc chunk at +256hc, m at +128m
O_EXPB = O_F2W + 2048           # [128, 8192]: pair (g,kc,p) at 1024*(4g+2kc+p)
O_NROW = O_EXPB + 8192          # row 0 only: [1, 4096] noise*strength (bf16)
C16 = O_NROW + 4096
# blob32 [128, C32] f32
O_XT32 = 0                      # [128, 8192]  x f32
O_BIAS = 8192                   # [128, 16]: cpb m at +m, apb +2+m, f2b +4+m,
C32 = O_BIAS + 16               #            f1b +6+hc (hc in 0..7)

EXPSET = 6      # natural_log_exp_and_others (has both Exp and Ln)
ACT_REMAP = {0: EXPSET, 5: EXPSET}   # exp_and_others / natural_log -> shared

_NC_CACHE = {}


def _rel_pos_index():
    c = np.stack(np.meshgrid(np.arange(WS), np.arange(WS), indexing="ij"))
    c = c.reshape(2, -1)
    rel = c[:, :, None] - c[:, None, :]
    rel = rel.transpose(1, 2, 0) + (WS - 1)
    return rel[..., 0] * (2 * WS - 1) + rel[..., 1]


def _perm():
    """t' (window-major) -> n (row-major within the core's 32x128 slab)."""
    t = np.arange(TOK)
    win, intra = t // 256, t % 256
    wr, wc = win // 8, win % 8
    rr, cc = intra // 16, intra % 16
    return (wr * 16 + rr) * 128 + (wc * 16 + cc)


def _patch_act_tables(nc):
    """Retarget Exp/Ln table loads to the shared natural_log_exp set and
    drop loads that are redundant on the (linear) instruction stream."""
    for b in nc.m.functions[0].blocks:
        insts = b.instructions
        cur = None
        i = 0
        while i < len(insts):
            inst = insts[i]
            if inst.opcode == "LoadActFuncSet":
                want = ACT_REMAP.get(inst.act_func_set_id,
                                     inst.act_func_set_id)
                if want == cur and inst.sync_info is None:
                    del insts[i]
                    continue
                inst.act_func_set_id = want
                cur = want
            i += 1


def build_nc(krep=1):
    nc = bacc.Bacc("TRN2", debug=False)
    blob16 = nc.dram_tensor("blob16", (128, C16), BF16, kind="ExternalInput")
    blob32 = nc.dram_tensor("blob32", (128, C32), F32, kind="ExternalInput")
    outT = nc.dram_tensor("outT", (DIM, TOK), F32, kind="ExternalOutput")

    EXP = mybir.ActivationFunctionType.Exp
    LN = mybir.ActivationFunctionType.Ln
    GELU = mybir.ActivationFunctionType.Gelu

    with tile.TileContext(nc) as tc:
        with (
            tc.tile_pool(name="wts", bufs=1) as wts,
            tc.tile_pool(name="x2p", bufs=2) as x2p,
            tc.tile_pool(name="work", bufs=2) as work,
            tc.tile_pool(name="attn", bufs=2) as attn,
            tc.tile_pool(name="ps", bufs=8, space="PSUM") as ps,
        ):
            def pst(name):
                return ps.tile([128, 512], F32, name=name, tag="bank")

            def wt(shape, name, dtype=F32, bufs=None, tag=None):
                kw_ = {"bufs": bufs} if bufs else {}
                return work.tile(list(shape), dtype, name=name,
                                 tag=tag or name, **kw_)

            # ---- resident weight loads (once, outside the rep loop) ----
            def load16(off, cols, name):
                s = wts.tile([128, cols], BF16, name=name)
                nc.sync.dma_start(out=s, in_=blob16.ap()[:, off:off + cols])
                return s

            s_emb = load16(O_EMBT, 512, "s_emb")
            s_w = {n: load16(o, 512, f"s_{n}") for n, o in O_W.items()}
            s_f1w = load16(O_F1W, 2048, "s_f1w")
            s_f2w = load16(O_F2W, 2048, "s_f2w")
            s_expb = load16(O_EXPB, 8192, "s_expb")
            s_b32 = wts.tile([128, 16], F32, name="s_b32")
            nc.sync.dma_start(out=s_b32, in_=blob32.ap()[:, O_BIAS:O_BIAS + 16])
            s_o32 = wts.tile([128, 32], BF16, name="s_o32")
            nc.vector.memset(s_o32, 1.0)
            s_on = wts.tile([128, 128], BF16, name="s_on")
            nc.vector.memset(s_on, 1.0 / DIM)
            s_eps = wts.tile([128, 1], F32, name="s_eps")
            nc.vector.memset(s_eps, 1e-8)

            def cpb(m):
                return s_b32[:, m:m + 1]

            def apb(m):
                return s_b32[:, 2 + m:3 + m]

            def f2b(m):
                return s_b32[:, 4 + m:5 + m]

            def f1b(hc):
                return s_b32[:, 6 + hc:7 + hc]

            def emit_rep():
                # ---- cross-attn K/V prep (embeddings are an input =>
                #      recompute per rep for honest per-forward timing) ----
                k_cm = [attn.tile([128, EN], BF16, name=f"kcm{m}",
                                  tag=f"kcm{m}") for m in range(2)]
                v_km = [attn.tile([128, DIM], BF16, name=f"vkm{m}",
                                  tag=f"vkm{m}") for m in range(2)]
                for m in range(2):
                    p = pst(f"kvp{m}")
                    for kc in range(2):
                        nc.tensor.matmul(
                            p[:, 0:EN],
                            s_w["ckw"][:, 256 * kc + 128 * m:256 * kc + 128 * (m + 1)],
                            s_emb[:, 256 * kc:256 * (kc + 1)],
                            start=(kc == 0), stop=(kc == 1))
                    nc.vector.tensor_copy(k_cm[m], p[:, 0:EN])
                    p2 = pst(f"vvp{m}")
                    for kc in range(2):
                        nc.tensor.matmul(
                            p2[:, 0:DIM],
                            s_emb[:, 256 * kc + 128 * m:256 * kc + 128 * (m + 1)],
                            s_w["cvw"][:, 256 * kc:256 * (kc + 1)],
                            start=(kc == 0), stop=(kc == 1))
                    nc.vector.tensor_copy(v_km[m], p2[:, 0:DIM])

                x2f = [x2p.tile([128, TOK], BF16, name=f"x2f{m}",
                                tag=f"x2f{m}") for m in range(2)]
                s2b = x2p.tile([128, TOK], BF16, name="s2b", tag="s2b",
                               bufs=1)

                # softmax tail: denominators + AV for one head-group g.
                # epair[kc][p] = [128,1024] bf16 exp tiles (head 2p+j at
                # cols 512j); av_cols(hl, kc) yields (colslice, lhsT) pairs.
                def attn_tail(g, ee, av_cols, pname):
                    sb = pst(f"{pname}sb{g}")
                    for hl in range(4):
                        for kc in range(2):
                            nc.tensor.matmul(
                                sb[32 * hl:32 * hl + 32, :],
                                s_o32[:, 0:32],
                                ee[kc][hl],
                                start=(kc == 0), stop=(kc == 1),
                                tile_position=(0, 32 * hl))
                    rb = wt([128, TC], f"rb{g}", F32, tag=f"rb{g}")
                    with tc.high_priority(offset=400):
                        nc.vector.reciprocal_approx_fast(out=rb, in_=sb)
                    ou = pst(f"{pname}ou{g}")
                    for hl in range(4):
                        for csl, lhsTs in av_cols(hl):
                            for kc in range(2):
                                nc.tensor.matmul(
                                    ou[32 * hl:32 * hl + 32, csl],
                                    lhsTs[kc],
                                    ee[kc][hl][:, csl],
                                    start=(kc == 0), stop=(kc == 1),
                                    tile_position=(0, 32 * hl))
                    on = attn.tile([128, TC], BF16, name=f"on{g}",
                                   tag=f"on{g}")
                    nc.vector.tensor_mul(on, ou, rb)
                    return on

                # =========== phase A: attention, per 512-token chunk =======
                # software-pipelined: cross(t+1) is emitted before
                # window(t) so TensorE has independent matmuls to run
                # during window(t)'s serial pixel-norm/softmax chains.
                def stage_cross(t):
                    # chunk input loads
                    x0 = [wt([128, TC], f"x0_{m}", F32, tag=f"x0_{m}")
                          for m in range(2)]
                    x16 = [work.tile([128, TC], BF16, name=f"x16_{m}",
                                     tag=f"x16_{m}") for m in range(2)]
                    for m in range(2):
                        nc.sync.dma_start(
                            out=x0[m],
                            in_=blob32.ap()[:, O_XT32 + 4096 * m + TC * t:
                                            O_XT32 + 4096 * m + TC * (t + 1)])
                        nc.sync.dma_start(
                            out=x16[m],
                            in_=blob16.ap()[:, O_XT16 + 4096 * m + TC * t:
                                            O_XT16 + 4096 * m + TC * (t + 1)])

                    # ---- cross attention ----
                    q_cm = [work.tile([128, TC], BF16, name=f"qcm{m}",
                                      tag=f"qcm{m}") for m in range(2)]
                    for m in range(2):
                        p = pst(f"qp{m}")
                        for kc in range(2):
                            nc.tensor.matmul(
                                p,
                                s_w["cqw"][:, 256 * kc + 128 * m:
                                           256 * kc + 128 * (m + 1)],
                                x16[kc], start=(kc == 0), stop=(kc == 1))
                        with tc.high_priority(offset=400):
                            nc.vector.tensor_copy(q_cm[m], p)
                    o_n = [None, None]
                    for g in range(2):
                        ee = [[None] * 4, [None] * 4]
                        for kc in range(2):
                            scs = []
                            for hl in range(4):
                                sc = pst(f"csc{g}{hl}{kc}")
                                nc.tensor.matmul(
                                    sc,
                                    k_cm[g][32 * hl:32 * hl + 32,
                                            128 * kc:128 * (kc + 1)],
                                    q_cm[g][32 * hl:32 * hl + 32, :],
                                    start=True, stop=True,
                                    tile_position=(32 * hl, 0))
                                scs.append(sc)
                            for hl in range(4):
                                e = attn.tile([128, TC], BF16,
                                              name=f"ce{hl}{kc}",
                                              tag=f"ce{hl}{kc}")
                                nc.scalar.activation(out=e, in_=scs[hl],
                                                     func=EXP)
                                ee[kc][hl] = e

                        def av_cols(hl, g=g):
                            hh = 4 * g + hl
                            return [(slice(0, TC),
                                     [v_km[kc][:, 32 * hh:32 * hh + 32]
                                      for kc in range(2)])]
                        o_n[g] = attn_tail(g, ee, av_cols, f"c{t}")
                    x1 = [wt([128, TC], f"x1_{m}", F32, tag=f"x1_{m}",
                              bufs=3) for m in range(2)]
                    for m in range(2):
                        p = pst(f"cp{m}")
                        for kc in range(2):
                            nc.tensor.matmul(
                                p,
                                s_w["cpw"][:, 256 * kc + 128 * m:
                                           256 * kc + 128 * (m + 1)],
                                o_n[kc], start=(kc == 0), stop=(kc == 1))
                        nc.vector._custom_dve(
                            AFFINE_THEN_ADD, out=x1[m], in0=p,
                            in1=x0[m], s0=1.0, s1=cpb(m))
                    return x1

                def stage_norm(t, x1):
                    nb = wt([128, TC], "nb", BF16, bufs=2)
                    nc.sync.dma_start(
                        out=nb,
                        in_=blob16.ap()[0:1, O_NROW + TC * t:O_NROW + TC * (t + 1)]
                        .to_broadcast([128, TC]))
                    # ---- pixel norm + noise ----
                    sq = [work.tile([128, TC], BF16, name=f"sq{m}",
                                    tag=f"sq{m}") for m in range(2)]
                    for m in range(2):
                        with tc.high_priority(offset=400):
                            nc.vector.tensor_mul(sq[m], x1[m], x1[m])
                    mb = pst("mb")
                    for kc in range(2):
                        nc.tensor.matmul(mb, s_on, sq[kc],
                                         start=(kc == 0), stop=(kc == 1))
                    rs = wt([128, TC], "rs", F32, bufs=2)
                    nc.scalar.activation(out=rs, in_=mb, func=LN, bias=s_eps)
                    nc.scalar.activation(out=rs, in_=rs, func=EXP, scale=-0.5)
                    xn = [work.tile([128, TC], BF16, name=f"xn{m}",
                                    tag=f"xn{m}") for m in range(2)]
                    for m in range(2):
                        xt = wt([128, TC], f"xt{m}", F32, bufs=2, tag=f"xt{m}")
                        nc.vector.tensor_mul(xt, x1[m], rs)
                        nc.vector.tensor_add(xn[m], xt, nb)
                    return xn

                def stage_winattn(t, x1, xn):
                    tsl = slice(TC * t, TC * (t + 1))
                    # ---- window attention ----
                    qk = {}
                    for m in range(2):
                        for wname in ("qw", "kw"):
                            p = pst(f"qk{wname}{m}")
                            for kc in range(2):
                                nc.tensor.matmul(
                                    p,
                                    s_w[wname][:, 256 * kc + 128 * m:
                                               256 * kc + 128 * (m + 1)],
                                    xn[kc], start=(kc == 0), stop=(kc == 1))
                            d = work.tile([128, TC], BF16, name=f"{wname}{m}",
                                          tag=f"{wname}{m}")
                            with tc.high_priority(offset=400):
                                nc.vector.tensor_copy(d, p)
                            qk[(wname, m)] = d
                    v_kmw = [[None] * 2 for _ in range(2)]
                    for wloc in range(2):
                        for kcw in range(2):
                            base = 256 * wloc + 128 * kcw
                            p = pst(f"vw{wloc}{kcw}")
                            for cc in range(2):
                                nc.tensor.matmul(
                                    p[:, 0:DIM], xn[cc][:, base:base + 128],
                                    s_w["vw"][:, 256 * cc:256 * (cc + 1)],
                                    start=(cc == 0), stop=(cc == 1))
                            v = attn.tile([128, DIM], BF16,
                                          name=f"vkw{wloc}{kcw}",
                                          tag=f"vkw{wloc}{kcw}")
                            nc.vector.tensor_copy(v, p[:, 0:DIM])
                            v_kmw[wloc][kcw] = v
                    o_n = [None, None]
                    for g in range(2):
                        ee = [[None] * 4, [None] * 4]
                        for kc in range(2):
                            scs = []
                            for hl in range(4):
                                sc = pst(f"wsc{g}{hl}{kc}")
                                for wloc in range(2):
                                    kbase = 256 * wloc + 128 * kc
                                    nc.tensor.matmul(
                                        sc[:, 256 * wloc:256 * (wloc + 1)],
                                        qk[("kw", g)][32 * hl:32 * hl + 32,
                                                      kbase:kbase + 128],
                                        qk[("qw", g)][32 * hl:32 * hl + 32,
                                                      256 * wloc:
                                                      256 * (wloc + 1)],
                                        start=True, stop=True,
                                        tile_position=(32 * hl, 0))
                                scs.append(sc)
                            for hl in range(4):
                                e = attn.tile([128, TC], BF16,
                                              name=f"we{hl}{kc}",
                                              tag=f"we{hl}{kc}")
                                nc.scalar.activation(out=e, in_=scs[hl],
                                                     func=EXP)
                                hh = 4 * g + hl
                                off = 1024 * (4 * g + 2 * kc + (hl // 2)) \
                                    + 512 * (hl % 2)
                                nc.vector.tensor_mul(
                                    e, e, s_expb[:, off:off + 512])
                                ee[kc][hl] = e

                        def av_cols(hl, g=g):
                            hh = 4 * g + hl
                            return [
                                (slice(256 * wloc, 256 * (wloc + 1)),
                                 [v_kmw[wloc][kc][:, 32 * hh:32 * hh + 32]
                                  for kc in range(2)])
                                for wloc in range(2)
                            ]
                        o_n[g] = attn_tail(g, ee, av_cols, f"w{t}")
                    for m in range(2):
                        p = pst(f"ap{m}")
                        for kc in range(2):
                            nc.tensor.matmul(
                                p,
                                s_w["apw"][:, 256 * kc + 128 * m:
                                           256 * kc + 128 * (m + 1)],
                                o_n[kc], start=(kc == 0), stop=(kc == 1))
                        nc.vector._custom_dve(
                            AFFINE_THEN_ADD, out=x2f[m][:, tsl], in0=p,
                            in1=x1[m], s0=1.0, s1=apb(m))

                    # ---- second pixel norm (factor stored for phase B) ----
                    msq = [work.tile([128, TC], BF16, name=f"msq{m}",
                                     tag=f"sq{m}") for m in range(2)]
                    for m in range(2):
                        nc.vector.tensor_mul(msq[m], x2f[m][:, tsl],
                                             x2f[m][:, tsl])
                    mb2 = pst("mb2")
                    for kc in range(2):
                        nc.tensor.matmul(mb2, s_on, msq[kc],
                                         start=(kc == 0), stop=(kc == 1))
                    lnv2 = wt([128, TC], "lnv2", F32, bufs=1)
                    nc.scalar.activation(out=lnv2, in_=mb2, func=LN,
                                         bias=s_eps)
                    nc.scalar.activation(out=s2b[:, tsl], in_=lnv2, func=EXP,
                                         scale=-0.5)

                x1s, xns = {}, {}
                x1s[0] = stage_cross(0)
                x1s[1] = stage_cross(1)
                xns[0] = stage_norm(0, x1s[0])
                for t in range(NTC):
                    if t + 2 < NTC:
                        x1s[t + 2] = stage_cross(t + 2)
                    if t + 1 < NTC:
                        xns[t + 1] = stage_norm(t + 1, x1s[t + 1])
                    stage_winattn(t, x1s[t], xns[t])

                # =========== phase B: MLP, per chunk ===========
                # reversed: chunk NTC-1's inputs are the last ready, so
                # the scheduler cannot hoist its gelus into phase A's ACT
                # stream (which would cost extra table-set reloads).
                for t in reversed(range(NTC)):
                    tsl = slice(TC * t, TC * (t + 1))
                    xn2 = [work.tile([128, TC], BF16, name=f"xn2{m}",
                                     tag=f"xn2{m}") for m in range(2)]
                    for m in range(2):
                        nc.vector.tensor_mul(xn2[m], x2f[m][:, tsl],
                                             s2b[:, tsl])
                    hsb = []
                    for hc in range(8):
                        p = pst(f"f1{hc}")
                        for kc in range(2):
                            nc.tensor.matmul(
                                p,
                                s_f1w[:, 1024 * kc + 128 * hc:
                                      1024 * kc + 128 * (hc + 1)],
                                xn2[kc], start=(kc == 0), stop=(kc == 1))
                        hh = work.tile([128, TC], BF16, name=f"h{hc}",
                                       tag=f"h{hc}", bufs=1)
                        nc.scalar.activation(out=hh, in_=p, func=GELU,
                                             bias=f1b(hc))
                        hsb.append(hh)
                    for m in range(2):
                        p = pst(f"f2{m}")
                        for hc in range(8):
                            nc.tensor.matmul(
                                p,
                                s_f2w[:, 256 * hc + 128 * m:
                                      256 * hc + 128 * (m + 1)],
                                hsb[hc], start=(hc == 0), stop=(hc == 7))
                        xo = wt([128, TC], f"xo{m}", F32, tag=f"xo{m}")
                        nc.vector._custom_dve(
                            AFFINE_THEN_ADD, out=xo, in0=p,
                            in1=x2f[m][:, tsl], s0=1.0, s1=f2b(m))
                        nc.gpsimd.dma_start(
                            out=outT.ap()[128 * m:128 * (m + 1), tsl], in_=xo)

            for _rep in range(krep):
                emit_rep()

    nc.compile()
    _patch_act_tables(nc)
    return nc


def _host_prep(x, embeddings, noise, cq_w, ck_w, cv_w, cp_w, cp_b,
               qkv_w, ap_w, ap_b, rpb_table, noise_strength,
               fc1_w, fc1_b, fc2_w, fc2_b):
    perm = _perm()
    idx = _rel_pos_index()
    bias = np.asarray(rpb_table)[idx.reshape(-1)].reshape(
        WS * WS, WS * WS, HEADS)
    biasT = np.exp(bias.transpose(2, 1, 0))   # exp! [h, key(m), tok(n)]
    expb = np.zeros((128, 8192), np.float32)
    for g in range(2):
        for kc in range(2):
            for p in range(2):
                off = 1024 * (4 * g + 2 * kc + p)
                for j in range(2):
                    h = 4 * g + 2 * p + j
                    blk = biasT[h, 128 * kc:128 * (kc + 1), :]  # [128, 256]
                    expb[:, off + 512 * j:off + 512 * j + 512] = (
                        np.concatenate([blk, blk], axis=1))
    f = np.asarray

    def w2(a):  # [256, X] -> [128, 2X] packed kc chunks side by side
        a = f(a)
        return np.concatenate([a[0:128], a[128:256]], axis=1)

    blob16_shared = np.concatenate([
        w2(f(cq_w) * SCALE), w2(ck_w), w2(cv_w), w2(cp_w),
        w2(f(qkv_w)[:, 0:DIM] * SCALE), w2(f(qkv_w)[:, DIM:2 * DIM]),
        w2(f(qkv_w)[:, 2 * DIM:3 * DIM]), w2(ap_w),
        w2(fc1_w),
        np.concatenate([np.concatenate(
            [f(fc2_w)[128 * hc:128 * (hc + 1), 0:128],
             f(fc2_w)[128 * hc:128 * (hc + 1), 128:256]], axis=1)
            for hc in range(8)], axis=1),
        expb,
    ], axis=1).astype(BF)

    bias32 = np.zeros((128, 16), np.float32)
    bias32[:, 0] = f(cp_b)[0:128]
    bias32[:, 1] = f(cp_b)[128:256]
    bias32[:, 2] = f(ap_b)[0:128]
    bias32[:, 3] = f(ap_b)[128:256]
    bias32[:, 4] = f(fc2_b)[0:128]
    bias32[:, 5] = f(fc2_b)[128:256]
    for hc in range(8):
        bias32[:, 6 + hc] = f(fc1_b)[128 * hc:128 * (hc + 1)]

    ins = []
    for c in range(NCORE):
        b, j = c // 4, c % 4
        xw = np.asarray(x)[b, TOK * j:TOK * (j + 1), :][perm]   # [4096, 256]
        xT = np.ascontiguousarray(xw.T)                          # [256, 4096]
        w0 = 64 * b + 16 * j
        nr = (np.asarray(noise)[w0:w0 + 16, :, 0].reshape(4096)
              * float(noise_strength)).astype(np.float32)
        embT = np.ascontiguousarray(np.asarray(embeddings)[b].T)  # [256, 256]
        nrow16 = np.zeros((128, 4096), BF)
        nrow16[0, :] = nr.astype(BF)
        blob16 = np.concatenate([
            np.concatenate([xT[0:128], xT[128:256]], axis=1).astype(BF),
            np.concatenate([embT[0:128], embT[128:256]], axis=1).astype(BF),
            blob16_shared,
            nrow16,
        ], axis=1)
        blob32 = np.zeros((128, C32), np.float32)
        blob32[:, 0:4096] = xT[0:128]
        blob32[:, 4096:8192] = xT[128:256]
        blob32[:, O_BIAS:O_BIAS + 16] = bias32
        ins.append({"blob16": blob16, "blob32": blob32})
    return ins, perm


def kernel(**inputs):
    global _NC_CACHE
    if 1 not in _NC_CACHE:
        _NC_CACHE[1] = build_nc(1)
    nc = _NC_CACHE[1]
    ins, perm = _host_prep(**inputs)
    res = run_bass_kernel_spmd(nc, ins, core_ids=list(range(NCORE)))
    inv = np.empty(TOK, np.int64)
    inv[perm] = np.arange(TOK)
    out = np.zeros((BS, N, DIM), np.float32)
    for c in range(NCORE):
        b, j = c // 4, c % 4
        oc = res.results[c]["outT"]                  # [256, 4096]
        out[b, TOK * j:TOK * (j + 1), :] = oc.T[inv]
    return out
